# revision 1
# baseline (speedup 1.0000x reference)
"""Trainium2 Bass kernel for nn_CrossAttentionT2S (fused pos-embed cross-attention).

Sharding: data-parallel over the true batch axis b=8, one batch element per
NeuronCore. All tensors on device are kept feature-major ("transposed",
[feature, token]) so every matmul contracts over the partition dimension with
no on-device transposes:

  sT  = s_xT + pos_sT                      [768, 1568]
  qT  = (q_w @ s)T * SCALE + q_b*SCALE     [768, 1568]   (feature-major)
  kT  = (k_w @ tp)T + k_b                  [768, 1568]
  V'  = (tp @ v_w.T + v_b | ones)          [1568, 12*65] (token-major, per-head
                                            64 cols + a ones column for the
                                            softmax denominator)
  per head h (paired 2 per 128-partition chunk, row-tiled on the PE):
    S^T[k, q] = (K_h)^T.T @ Q_h^T          PSUM [k_tile, q_tile]
    expS = exp(S^T)                        (ScalarE, PSUM->SBUF)
    O~T/den = V'_h.T @ expS                PSUM [65, q_tile], accum over k
    OT_h = O~T * (1/den)                   (DVE mul, denominator broadcast)
  outT = (proj_w @ O)T + proj_b            [768, 1568]

Host side does only layout prep (slicing/transposing numpy arrays) and the
inverse gather on the way out.
"""
import os
import sys
import types
from contextlib import ExitStack

import numpy as np

import concourse.bass as bass
import concourse.mybir as mybir
import concourse.tile as tile
from concourse import bacc
from concourse.bass_utils import run_bass_kernel_spmd

# ---------------------------------------------------------------- constants
DIM = 768
H = 12
DH = 64
T = 8
TS = 8
APATCH = 196
VP = 196
B = 8
NT = APATCH * TS          # 1568 tokens per core, both q and kv side
SCALE = DH ** -0.5
NCH = DIM // 128          # 6 feature chunks
KT = (NT + 127) // 128    # 13 k tiles (12 full + 32)
QS = 392                  # q slice (quarter) — 4 * 392 = 1568
F32 = mybir.dt.float32
F32R = mybir.dt.float32r
BF16 = mybir.dt.bfloat16

_NC_CACHE = {}


def _r(ap):
    return ap


def build_nc():
    nc = bacc.Bacc(None)

    s_xT = nc.dram_tensor("s_xT", [DIM, NT], F32R, kind="ExternalInput")
    t_xT = nc.dram_tensor("t_xT", [DIM, NT], F32R, kind="ExternalInput")
    pos_sT = nc.dram_tensor("pos_sT", [DIM, NT], F32R, kind="ExternalInput")
    pos_tT = nc.dram_tensor("pos_tT", [DIM, NT], F32R, kind="ExternalInput")
    q_wT = nc.dram_tensor("q_wT", [DIM, DIM], F32R, kind="ExternalInput")
    kv_wT = nc.dram_tensor("kv_wT", [DIM, 2 * DIM], F32R, kind="ExternalInput")
    proj_wT = nc.dram_tensor("proj_wT", [DIM, DIM], F32R, kind="ExternalInput")
    # biases as [128, NCH] (column c = per-partition bias of feature chunk c)
    q_b2 = nc.dram_tensor("q_b2", [128, NCH], F32, kind="ExternalInput")
    k_b2 = nc.dram_tensor("k_b2", [128, NCH], F32, kind="ExternalInput")
    v_br = nc.dram_tensor("v_br", [128, DIM], F32, kind="ExternalInput")
    p_b2 = nc.dram_tensor("p_b2", [128, NCH], F32, kind="ExternalInput")
    ones_h = nc.dram_tensor("ones_h", [128, H], BF16, kind="ExternalInput")
    outT = nc.dram_tensor("outT", [DIM, NT], F32, kind="ExternalOutput")

    with tile.TileContext(nc) as tc, ExitStack() as top:
        cpool = top.enter_context(tc.tile_pool(name="consts", bufs=1))
        qb_t = cpool.tile([128, NCH], F32, tag="qb")
        nc.gpsimd.dma_start(qb_t[:], q_b2[:])
        kb_t = cpool.tile([128, NCH], F32, tag="kb")
        nc.gpsimd.dma_start(kb_t[:], k_b2[:])
        pb_t = cpool.tile([128, NCH], F32, tag="pb")
        nc.gpsimd.dma_start(pb_t[:], p_b2[:])
        vb_t = cpool.tile([128, DIM], F32, tag="vb")
        nc.gpsimd.dma_start(vb_t[:], v_br[:, :])

        qT_pool = top.enter_context(tc.tile_pool(name="qT", bufs=NCH))
        kT_pool = top.enter_context(tc.tile_pool(name="kT", bufs=NCH))
        vP_pool = top.enter_context(tc.tile_pool(name="vP", bufs=KT))
        qT = [qT_pool.tile([128, NT], BF16, tag="qT", name=f"qT{i}") for i in range(NCH)]
        kT = [kT_pool.tile([128, NT], BF16, tag="kT", name=f"kT{i}") for i in range(NCH)]
        vP = [vP_pool.tile([128, H * (DH + 1)], BF16, tag="vP", name=f"vP{i}") for i in range(KT)]

        # ---------------- phase 1: inputs + positional adds + projections
        with ExitStack() as ph1, nc.named_scope("p1_qkv"):
            xT_pool = ph1.enter_context(tc.tile_pool(name="xT", bufs=12))
            pos_pool = ph1.enter_context(tc.tile_pool(name="pos", bufs=2))
            w_pool = ph1.enter_context(tc.tile_pool(name="w", bufs=NCH + 1))
            pj_psum = ph1.enter_context(
                tc.tile_pool(name="pj", bufs=2, space="PSUM")
            )

            def load_x(dram_x, dram_pos):
                xs = []
                for c in range(NCH):
                    t = xT_pool.tile([128, NT], F32R, tag="xT")
                    nc.sync.dma_start(t[:], dram_x[c * 128:(c + 1) * 128, :])
                    p = pos_pool.tile([128, NT], F32R, tag="pos")
                    nc.sync.dma_start(p[:], dram_pos[c * 128:(c + 1) * 128, :])
                    nc.vector.tensor_add(t[:], t[:], p[:])
                    xs.append(t)
                return xs

            def proj_fmajor(xs, w_dram, w_cols, out_tiles, bias_t, scale):
                """out[o, tok] = sum_d w[d, o] x[d, tok] (+bias)*scale."""
                ws = []
                for c in range(NCH):
                    wt = w_pool.tile([128, DIM], F32R, tag="w")
                    nc.scalar.dma_start(
                        wt[:], w_dram[c * 128:(c + 1) * 128, w_cols]
                    )
                    ws.append(wt)
                for ot in range(NCH):
                    ps = pj_psum.tile([128, 2048], F32, tag="pj")
                    for sl in range(4):
                        qsl = slice(sl * QS, (sl + 1) * QS)
                        psl = slice(sl * 512, sl * 512 + QS)
                        for c in range(NCH):
                            nc.tensor.matmul(
                                ps[:, psl],
                                _r(ws[c][:, ot * 128:(ot + 1) * 128]),
                                _r(xs[c][:, qsl]),
                                start=(c == 0),
                                stop=(c == NCH - 1),
                            )
                    nc.scalar.activation(
                        out_tiles[ot][:].rearrange("p (s q) -> p s q", s=4),
                        ps[:].rearrange("p (s q) -> p s q", s=4)[:, :, 0:QS],
                        mybir.ActivationFunctionType.Identity,
                        bias=bias_t[:, ot:ot + 1],
                        scale=scale,
                    )

            tT = load_x(t_xT, pos_tT)
            proj_fmajor(tT, kv_wT, slice(0, DIM), kT, kb_t, 1.0)
            sT = load_x(s_xT, pos_sT)
            proj_fmajor(sT, q_wT, slice(0, DIM), qT, qb_t, SCALE)

            # V token-major with per-head ones column
            vw = []
            for c in range(NCH):
                wt = w_pool.tile([128, DIM], F32R, tag="w")
                nc.scalar.dma_start(
                    wt[:], kv_wT[c * 128:(c + 1) * 128, DIM:2 * DIM]
                )
                vw.append(wt)
            for kt in range(KT):
                kw = min(128, NT - kt * 128)
                ps = pj_psum.tile([128, 2048], F32, tag="pj")
                for vs in range(2):
                    vsl = slice(vs * 384, (vs + 1) * 384)
                    psl = slice(vs * 512, vs * 512 + 384)
                    for c in range(NCH):
                        nc.tensor.matmul(
                            ps[:kw, psl],
                            _r(tT[c][:, kt * 128:kt * 128 + kw]),
                            _r(vw[c][:, vsl]),
                            start=(c == 0),
                            stop=(c == NCH - 1),
                        )
                vt = vP[kt]
                dst = vt[:kw, :].rearrange("p (v g c) -> p v g c", v=2, c=DH + 1)
                nc.vector.tensor_add(
                    dst[:, :, :, 0:DH],
                    ps[:kw, 0:1024].rearrange("p (v r) -> p v r", v=2)[:, :, 0:384]
                        .rearrange("p v (g c) -> p v g c", c=DH),
                    vb_t[:kw, :].rearrange("p (v g c) -> p v g c", v=2, c=DH),
                )
                dst3 = vt[:kw, :].rearrange("p (h c) -> p h c", c=DH + 1)
                nc.gpsimd.dma_start(dst3[:, :, DH:DH + 1], ones_h[:kw, :, None])

        # ---------------- phase 2: attention
        OT_pool = top.enter_context(tc.tile_pool(name="OT", bufs=NCH))
        OT = [OT_pool.tile([128, NT], F32R, tag="OT", name=f"OT{i}") for i in range(NCH)]
        with ExitStack() as ph2, nc.named_scope("p2_attn"):
            qk_psum = ph2.enter_context(
                tc.tile_pool(name="qk", bufs=2, space="PSUM")
            )
            o_psum = ph2.enter_context(
                tc.tile_pool(name="ops", bufs=4, space="PSUM")
            )
            exp_pool = ph2.enter_context(tc.tile_pool(name="expS", bufs=3))
            dcol_pool = ph2.enter_context(tc.tile_pool(name="dcol", bufs=2))
            rcp_pool = ph2.enter_context(tc.tile_pool(name="rcp", bufs=2))
            rdb_pool = ph2.enter_context(tc.tile_pool(name="rdenb", bufs=3))
            tmp_pool = ph2.enter_context(tc.tile_pool(name="otmp", bufs=2))

            for c6 in range(NCH):
                def qk_mm(qk, ki, qsl):
                    kw = min(128, NT - ki * 128)
                    ksl = slice(ki * 128, ki * 128 + kw)
                    nc.tensor.matmul(
                        qk[:kw, 0:QS],
                        kT[c6][0:64, ksl],
                        qT[c6][0:64, qsl],
                        start=True, stop=True,
                        tile_position=(0, 0),
                    )
                    nc.tensor.matmul(
                        qk[:kw, 512:512 + QS],
                        kT[c6][64:128, ksl],
                        qT[c6][64:128, qsl],
                        start=True, stop=True,
                        tile_position=(64, 0),
                    )

                for qt in range(4):
                    qsl = slice(qt * QS, (qt + 1) * QS)
                    ops = [o_psum.tile([DH + 1, QS], F32, tag="ops", name=f"ops{i}") for i in range(2)]
                    qk_next = qk_psum.tile([128, 1024], F32, tag="qk", name="qk0")
                    qk_mm(qk_next, 0, qsl)
                    for ki in range(KT):
                        kw = min(128, NT - ki * 128)
                        qk = qk_next
                        if ki + 1 < KT:
                            qk_next = qk_psum.tile([128, 1024], F32, tag="qk",
                                                   name=f"qk{ki+1}")
                            qk_mm(qk_next, ki + 1, qsl)
                        ex = exp_pool.tile([128, 2 * QS], BF16, tag="expS")
                        nc.scalar.activation(
                            ex[:kw, :].rearrange("p (b q) -> p b q", b=2),
                            qk[:kw, :].rearrange("p (b q) -> p b q", b=2)[:, :, 0:QS],
                            mybir.ActivationFunctionType.Exp,
                        )
                        for hh in range(2):
                            h = 2 * c6 + hh
                            nc.tensor.matmul(
                                ops[hh][:, :],
                                vP[ki][:kw, h * (DH + 1):(h + 1) * (DH + 1)],
                                ex[:kw, hh * QS:(hh + 1) * QS],
                                start=(ki == 0),
                                stop=(ki == KT - 1),
                            )
                    # normalize + evacuate
                    for hh in range(2):
                        rden = rcp_pool.tile([1, QS], F32, tag="rcp")
                        nc.vector.reciprocal(rden[:], ops[hh][DH:DH + 1, :])
                        rdb = rdb_pool.tile([64, QS], F32, tag="rdenb")
                        nc.gpsimd.partition_broadcast(rdb[:], rden[:, :])
                        if hh == 0:
                            nc.vector.tensor_mul(
                                OT[c6][0:DH, qsl], ops[hh][0:DH, :], rdb[:]
                            )
                        else:
                            tmp = tmp_pool.tile([64, QS], F32R, tag="otmp")
                            nc.vector.tensor_mul(tmp[:], ops[hh][0:DH, :], rdb[:])
                            nc.sync.dma_start(OT[c6][64:128, qsl], tmp[:])

        # ---------------- phase 3: output projection
        with ExitStack() as ph3, nc.named_scope("p3_proj"):
            pw_pool = ph3.enter_context(tc.tile_pool(name="pw", bufs=NCH))
            pj2_psum = ph3.enter_context(
                tc.tile_pool(name="pj2", bufs=2, space="PSUM")
            )
            oev_pool = ph3.enter_context(tc.tile_pool(name="oev", bufs=2))
            pw = []
            for c in range(NCH):
                wt = pw_pool.tile([128, DIM], F32R, tag="pw", name=f"pw{c}")
                nc.sync.dma_start(wt[:], proj_wT[c * 128:(c + 1) * 128, :])
                pw.append(wt)
            for ot in range(NCH):
                ps = pj2_psum.tile([128, 2048], F32, tag="pj2")
                for sl in range(4):
                    qsl = slice(sl * QS, (sl + 1) * QS)
                    psl = slice(sl * 512, sl * 512 + QS)
                    for c in range(NCH):
                        nc.tensor.matmul(
                            ps[:, psl],
                            _r(pw[c][:, ot * 128:(ot + 1) * 128]),
                            _r(OT[c][:, qsl]),
                            start=(c == 0),
                            stop=(c == NCH - 1),
                        )
                oe = oev_pool.tile([128, NT], F32, tag="oev")
                nc.scalar.activation(
                    oe[:].rearrange("p (s q) -> p s q", s=4),
                    ps[:].rearrange("p (s q) -> p s q", s=4)[:, :, 0:QS],
                    mybir.ActivationFunctionType.Identity,
                    bias=pb_t[:, ot:ot + 1],
                    scale=1.0,
                )
                nc.sync.dma_start(outT[ot * 128:(ot + 1) * 128, :], oe[:])

    nc.finalize()
    return nc


def _install_axon_ntff_shim():
    if "antenv.axon_hooks" in sys.modules:
        return
    mod = types.ModuleType("antenv.axon_hooks")
    mod._hook = None
    mod.set_axon_ntff_profile_hook = lambda h: setattr(mod, "_hook", h)
    mod.get_axon_ntff_profile_hook = lambda: mod._hook
    sys.modules["antenv.axon_hooks"] = mod
    try:
        import antenv

        antenv.axon_hooks = mod
        from trn_agent_boot.trn_boot import _ntff_profile_via_ctypes

        hook = _ntff_profile_via_ctypes("/opt/axon/libaxon_pjrt.so")
        if hook is not None:
            mod.set_axon_ntff_profile_hook(hook)
    except Exception:
        pass


def prep_inputs(s_x, t_x, clip_space_pos, vmae_space_pos, clip_temporal_pos,
                vmae_temporal_pos, q_w, q_b, kv_w, kv_b, proj_w, proj_b):
    """Host-side sharding/layout prep. Returns list of 8 per-core input maps."""
    f = np.float32
    pos_sT = np.ascontiguousarray(
        (clip_space_pos.T[:, :, None] + clip_temporal_pos.T[:, None, :])
        .reshape(DIM, NT), dtype=f)
    pos_tT = np.ascontiguousarray(
        (vmae_space_pos.T[:, :, None] + vmae_temporal_pos.T[:, None, :])
        .reshape(DIM, NT), dtype=f)
    q_wT = np.ascontiguousarray(q_w.T, dtype=f)
    kv_wT = np.ascontiguousarray(kv_w.T, dtype=f)
    proj_wT = np.ascontiguousarray(proj_w.T, dtype=f)
    q_b2 = np.ascontiguousarray((q_b * SCALE).reshape(NCH, 128).T, dtype=f)
    k_b2 = np.ascontiguousarray(kv_b[:DIM].reshape(NCH, 128).T, dtype=f)
    v_br = np.ascontiguousarray(np.broadcast_to(kv_b[DIM:].reshape(1, DIM), (128, DIM)), dtype=f)
    p_b2 = np.ascontiguousarray(proj_b.reshape(NCH, 128).T, dtype=f)
    import ml_dtypes
    ones_h = np.ones((128, H), dtype=ml_dtypes.bfloat16)

    in_maps = []
    for b in range(B):
        s_slice = s_x[:, b * TS:(b + 1) * TS, :]       # (196, 8, 768)
        t_slice = t_x[1:, b * T:(b + 1) * T, :]        # (196, 8, 768)
        s_xT = np.ascontiguousarray(
            s_slice.transpose(2, 0, 1).reshape(DIM, NT), dtype=f)
        t_xT = np.ascontiguousarray(
            t_slice.transpose(2, 0, 1).reshape(DIM, NT), dtype=f)
        in_maps.append({
            "s_xT": s_xT, "t_xT": t_xT,
            "pos_sT": pos_sT, "pos_tT": pos_tT,
            "q_wT": q_wT, "kv_wT": kv_wT, "proj_wT": proj_wT,
            "q_b2": q_b2, "k_b2": k_b2, "v_br": v_br, "p_b2": p_b2,
            "ones_h": ones_h,
        })
    return in_maps


def unshard_output(results):
    """results: list of 8 dicts with 'outT' [768, 1568] -> (196, 64, 768)."""
    out = np.empty((APATCH, B * TS, DIM), dtype=np.float32)
    for b in range(B):
        # outT[d, n*TS+t] -> out[n, b*TS+t, d]
        o = results[b]["outT"].reshape(DIM, APATCH, TS)
        out[:, b * TS:(b + 1) * TS, :] = o.transpose(1, 2, 0)
    return out


def kernel(**inputs):
    _install_axon_ntff_shim()
    in_maps = prep_inputs(**inputs)
    if "nc" not in _NC_CACHE:
        _NC_CACHE["nc"] = build_nc()
    nc = _NC_CACHE["nc"]
    res = run_bass_kernel_spmd(nc, in_maps, core_ids=list(range(B)))
    return unshard_output(res.results)


if __name__ == "__main__":
    rng = np.random.default_rng(0)
    fake = {
        "s_x": rng.standard_normal((APATCH, B * TS, DIM), dtype=np.float32),
        "t_x": rng.standard_normal((VP + 1, B * T, DIM), dtype=np.float32),
        "clip_space_pos": SCALE * rng.standard_normal((APATCH, DIM), dtype=np.float32),
        "vmae_space_pos": SCALE * rng.standard_normal((VP, DIM), dtype=np.float32),
        "clip_temporal_pos": SCALE * rng.standard_normal((TS, DIM), dtype=np.float32),
        "vmae_temporal_pos": SCALE * rng.standard_normal((T, DIM), dtype=np.float32),
        "q_w": (0.02 * rng.standard_normal((DIM, DIM))).astype(np.float32),
        "q_b": np.zeros(DIM, np.float32),
        "kv_w": (0.02 * rng.standard_normal((2 * DIM, DIM))).astype(np.float32),
        "kv_b": np.zeros(2 * DIM, np.float32),
        "proj_w": (0.02 * rng.standard_normal((DIM, DIM))).astype(np.float32),
        "proj_b": np.zeros(DIM, np.float32),
    }
    out = kernel(**fake)
    print("out", out.shape, out.dtype)



# revision 14
# speedup vs baseline: 1.4346x; 1.4346x over previous
"""Trainium2 Bass kernel for nn_CrossAttentionT2S (fused pos-embed cross-attention).

Sharding: data-parallel over the true batch axis b=8, one batch element per
NeuronCore. All tensors bf16 on device; feature-major ("transposed",
[feature, token]) layouts so matmuls contract over the partition dim.

Per core (NT=1568 q tokens, 1568 kv tokens, 12 heads, dh=64):
  tT = t_xT + pos_tT ; sT = s_xT + pos_sT          (DVE bf16 adds)
  kT = k_w @ t (feature-major, 6 chunks of 2 heads) (PE, evac on DVE + bias)
  qT = (q_w*SCALE) @ s + q_b*SCALE                  (PE, evac DVE)
  V' = token-major [128tok, parity, 6, 128]: even heads [v|ones64],
       odd heads [ones64|v]                          (PE, evac DVE, ones memset)
  per (qb in 512,512,512 + rump32, c6 head-pair, ki in 13):
    S[k128, q512]x2 heads — two row-tiled matmuls (0,0)/(64,0), concurrent
    P = exp(S) — ONE ScalarE activation per (c6,qb,ki), [128, 1024] free
    O~/den: AV matmul lhsT=V'[h] M=128: 64 cols of v + 64 ones columns ->
       psum [128,512]: O~ on one 64-partition half, den replicated on other
    evac: DVE reciprocal_approx_fast(den half) -> rcp, DVE mul -> OT bf16
  out = proj_w @ O + proj_b (PE, interleaved with next qb; DVE evac, DMA out)

ScalarE runs ONLY exp (the structural bottleneck ~275us); everything else is
kept off it. Projections/out-proj are emitted interleaved with attention so
the PE fills its exp-wait gaps and ACT never idles after warmup.
"""
import sys
import types
from contextlib import ExitStack

import numpy as np
import ml_dtypes

import concourse.bass as bass
import concourse.mybir as mybir
import concourse.tile as tile
from concourse import bacc
from concourse.bass_utils import run_bass_kernel_spmd

# ---------------------------------------------------------------- constants
DIM = 768
H = 12
DH = 64
T = 8
TS = 8
APATCH = 196
VP = 196
B = 8
NT = APATCH * TS          # 1568 tokens per core, both q and kv side
SCALE = DH ** -0.5
NCH = DIM // 128          # 6 feature chunks (2 heads each)
KT = 13                   # k tiles: 12 full 128 + rump 32
KR = NT - 12 * 128        # 32
QB = 512                  # q block
NQB = 3                   # full q blocks; rump = 32
QR = NT - NQB * QB        # 32
F32 = mybir.dt.float32
BF16 = mybir.dt.bfloat16
ADD = mybir.AluOpType.add
MULT = mybir.AluOpType.mult

_NC_CACHE = {}

import os
V_RECIP = os.environ.get("KV_RECIP", "fast")     # fast | exact
V_MEMSET = os.environ.get("KV_MEMSET", "pool")   # pool | dve
V_RUMP = os.environ.get("KV_RUMP", "1") == "1"
V_QALL = os.environ.get("KV_QALL", "0") == "1"
V_SERIAL = os.environ.get("KV_SERIAL", "0") == "1"



def kw_of(ki):
    return 128 if ki < 12 else KR


def build_nc():
    nc = bacc.Bacc(None)

    s_xT = nc.dram_tensor("s_xT", [DIM, NT], BF16, kind="ExternalInput")
    t_xT = nc.dram_tensor("t_xT", [DIM, NT], BF16, kind="ExternalInput")
    pos_sT = nc.dram_tensor("pos_sT", [DIM, NT], BF16, kind="ExternalInput")
    pos_tT = nc.dram_tensor("pos_tT", [DIM, NT], BF16, kind="ExternalInput")
    q_wT = nc.dram_tensor("q_wT", [DIM, DIM], BF16, kind="ExternalInput")
    k_wT = nc.dram_tensor("k_wT", [DIM, DIM], BF16, kind="ExternalInput")
    v_wT = nc.dram_tensor("v_wT", [DIM, DIM], BF16, kind="ExternalInput")
    proj_wT = nc.dram_tensor("proj_wT", [DIM, DIM], BF16, kind="ExternalInput")
    q_b2 = nc.dram_tensor("q_b2", [128, NCH], F32, kind="ExternalInput")
    k_b2 = nc.dram_tensor("k_b2", [128, NCH], F32, kind="ExternalInput")
    p_b2 = nc.dram_tensor("p_b2", [128, NCH], F32, kind="ExternalInput")
    v_br = nc.dram_tensor("v_br", [128, DIM], F32, kind="ExternalInput")
    outT = nc.dram_tensor("outT", [DIM, NT], F32, kind="ExternalOutput")
    DBG = os.environ.get("KV_DBG", "0") == "1"
    if DBG:
        dbg_qT = nc.dram_tensor("dbg_qT", [NCH, 128, NT], BF16, kind="ExternalOutput")
        dbg_kT = nc.dram_tensor("dbg_kT", [128, NT], BF16, kind="ExternalOutput")
        dbg_vP = nc.dram_tensor("dbg_vP", [128, 2, NCH, 128], BF16, kind="ExternalOutput")
        dbg_OT = nc.dram_tensor("dbg_OT", [NCH, 128, NT], BF16, kind="ExternalOutput")
        dbg_P = nc.dram_tensor("dbg_P", [128, 2, 512], BF16, kind="ExternalOutput")
        dbg_sT = nc.dram_tensor("dbg_sT", [128, NT], BF16, kind="ExternalOutput")

    with tile.TileContext(nc) as tc, ExitStack() as top:
        # ---------------- constant / persistent tiles
        cpool = top.enter_context(tc.tile_pool(name="consts", bufs=1))
        qb_t = cpool.tile([128, NCH], F32, tag="qb")
        kb_t = cpool.tile([128, NCH], F32, tag="kb")
        pb_t = cpool.tile([128, NCH], F32, tag="pb")
        vb_t = cpool.tile([128, DIM], F32, tag="vb")
        nc.sync.dma_start(qb_t[:], q_b2[:])
        nc.sync.dma_start(kb_t[:], k_b2[:])
        nc.sync.dma_start(pb_t[:], p_b2[:])
        nc.gpsimd.dma_start(vb_t[:], v_br[:])

        w_pool = top.enter_context(tc.tile_pool(name="w", bufs=NCH))
        qw = [w_pool.tile([128, DIM], BF16, tag="qw", name=f"qw{c}") for c in range(NCH)]
        kw = [w_pool.tile([128, DIM], BF16, tag="kw", name=f"kw{c}") for c in range(NCH)]
        vw = [w_pool.tile([128, DIM], BF16, tag="vw", name=f"vw{c}") for c in range(NCH)]
        pw = [w_pool.tile([128, DIM], BF16, tag="pw", name=f"pw{c}") for c in range(NCH)]
        for c in range(NCH):
            sl = slice(c * 128, (c + 1) * 128)
            nc.sync.dma_start(kw[c][:], k_wT[sl, :])
            nc.gpsimd.dma_start(vw[c][:], v_wT[sl, :])
            nc.sync.dma_start(qw[c][:], q_wT[sl, :])
            nc.gpsimd.dma_start(pw[c][:], proj_wT[sl, :])

        # x + pos, bf16 feature-major
        xs_pool = top.enter_context(tc.tile_pool(name="xs", bufs=NCH))
        sT = [xs_pool.tile([128, NT], BF16, tag="sT", name=f"sT{c}") for c in range(NCH)]
        tT = [xs_pool.tile([128, NT], BF16, tag="tT", name=f"tT{c}") for c in range(NCH)]
        with ExitStack() as pr, nc.named_scope("p0_load"):
            xin_pool = pr.enter_context(tc.tile_pool(name="xin", bufs=4))
            pos_pool = pr.enter_context(tc.tile_pool(name="pos", bufs=4))
            for c in range(NCH):
                sl = slice(c * 128, (c + 1) * 128)
                xt = xin_pool.tile([128, NT], BF16, tag="xin")
                pt = pos_pool.tile([128, NT], BF16, tag="pos")
                nc.gpsimd.dma_start(xt[:], t_xT[sl, :])
                nc.gpsimd.dma_start(pt[:], pos_tT[sl, :])
                nc.vector.tensor_add(tT[c][:], xt[:], pt[:])
                xs2 = xin_pool.tile([128, NT], BF16, tag="xin")
                ps2 = pos_pool.tile([128, NT], BF16, tag="pos")
                nc.sync.dma_start(xs2[:], s_xT[sl, :])
                nc.sync.dma_start(ps2[:], pos_sT[sl, :])
                nc.vector.tensor_add(sT[c][:], xs2[:], ps2[:])

        # q/k feature-major bf16; V' token-major bf16 with ones blocks
        qkT_pool = top.enter_context(tc.tile_pool(name="qkT", bufs=NCH))
        qT = [qkT_pool.tile([128, NT], BF16, tag="qT", name=f"qT{c}") for c in range(NCH)]
        kT = [qkT_pool.tile([128, NT], BF16, tag="kT", name=f"kT{c}") for c in range(NCH)]
        vP_pool = top.enter_context(tc.tile_pool(name="vP", bufs=KT))
        # layout: [tok, parity, pair, 128]; head h = 2*pair+parity
        vP = [vP_pool.tile([128, 2, NCH, 128], BF16, tag="vP", name=f"vP{k}")
              for k in range(KT)]
        ms_eng = nc.gpsimd if V_MEMSET == "pool" else nc.vector
        for k in range(KT):
            kwid = kw_of(k)
            ms_eng.memset(vP[k][:kwid, :, :, 0:64], 1.0)

        OT_pool = top.enter_context(tc.tile_pool(name="OT", bufs=NCH))
        OT = [OT_pool.tile([128, NT], BF16, tag="OT", name=f"OT{c}") for c in range(NCH)]

        # ---------------- psum pools (8 banks total)
        qk_psum = top.enter_context(tc.tile_pool(name="qkps", bufs=2, space="PSUM"))
        av_psum = top.enter_context(tc.tile_pool(name="avps", bufs=2, space="PSUM"))
        gm_psum = top.enter_context(tc.tile_pool(name="gmps", bufs=2, space="PSUM"))

        P_pool = top.enter_context(tc.tile_pool(name="P", bufs=3))
        rcp_pool = top.enter_context(tc.tile_pool(name="rcp", bufs=2))
        ost_pool = top.enter_context(tc.tile_pool(name="ost", bufs=2))

        QSL = [(i * QB, QB) for i in range(NQB)] + [(NQB * QB, QR)]

        def emit_proj(ws, xsrc, dst, bias_t, c_out, q0, qn):
            """dst[c_out][:, q0:q0+qn] = ws.T @ x (+bias), bf16 evac on DVE."""
            ps = gm_psum.tile([128, 512], F32, tag="gm")
            for c in range(NCH):
                nc.tensor.matmul(
                    ps[:, 0:qn],
                    ws[c][:, c_out * 128:(c_out + 1) * 128],
                    xsrc[c][:, q0:q0 + qn],
                    start=(c == 0), stop=(c == NCH - 1),
                )
            nc.vector.tensor_scalar_add(
                dst[c_out][:, q0:q0 + qn], ps[:, 0:qn], bias_t[:, c_out:c_out + 1]
            )

        def emit_vproj(k):
            """V' for k-tile k. v_wT cols pre-reordered on host:
            group0 = even heads' v dims, group1 = odd heads'."""
            kwid = kw_of(k)
            for g in range(2):
                ps = gm_psum.tile([128, 512], F32, tag="gm")
                for c in range(NCH):
                    nc.tensor.matmul(
                        ps[:kwid, 0:384],
                        tT[c][:, k * 128:k * 128 + kwid],
                        vw[c][:, g * 384:(g + 1) * 384],
                        start=(c == 0), stop=(c == NCH - 1),
                    )
                dst = vP[k][:kwid, g, :, 64:128]
                src = ps[:kwid, 0:384].rearrange("p (h d) -> p h d", d=DH)
                bia = vb_t[:kwid, g * 384:(g + 1) * 384].rearrange(
                    "p (h d) -> p h d", d=DH)
                nc.vector.tensor_tensor(dst, src, bia, ADD)

        def emit_outproj(c_out, q0, qn):
            ps = gm_psum.tile([128, 512], F32, tag="gm")
            for c in range(NCH):
                nc.tensor.matmul(
                    ps[:, 0:qn],
                    pw[c][:, c_out * 128:(c_out + 1) * 128],
                    OT[c][:, q0:q0 + qn],
                    start=(c == 0), stop=(c == NCH - 1),
                )
            oe = ost_pool.tile([128, 512], F32, tag="ost")
            nc.vector.tensor_scalar_add(
                oe[:, 0:qn], ps[:, 0:qn], pb_t[:, c_out:c_out + 1]
            )
            nc.sync.dma_start(outT[c_out * 128:(c_out + 1) * 128, q0:q0 + qn],
                              oe[:, 0:qn])

        filler = []  # deferred out-proj emissions (no forward PE deps)

        def filler_emit(n):
            for _ in range(n):
                if filler:
                    filler.pop(0)()

        def av_evac(c6, avA, avB, q0, qn):
            """Normalize + evac both heads of chunk c6 for q slice [q0, q0+qn)."""
            # both heads: den replicated at psum parts 0:64 (base-0 for the
            # custom DVE recip), O~ at 64:128; rcp written at base-0 SBUF.
            rcp = rcp_pool.tile([128, 1024], F32, tag="rcp")
            recip = (nc.vector.reciprocal_approx_fast if V_RECIP == "fast"
                     else nc.vector.reciprocal)
            recip(rcp[0:64, 0:qn], avA[0:64, 0:qn])
            recip(rcp[0:64, 512:512 + qn], avB[0:64, 0:qn])
            nc.vector.tensor_tensor(
                OT[c6][0:64, q0:q0 + qn], avA[64:128, 0:qn], rcp[0:64, 0:qn],
                MULT)
            nc.vector.tensor_tensor(
                OT[c6][64:128, q0:q0 + qn], avB[64:128, 0:qn],
                rcp[0:64, 512:512 + qn], MULT)

        def qk_mm(c6, ki, q0, qn):
            kwid = kw_of(ki)
            ksl = slice(ki * 128, ki * 128 + kwid)
            qk = qk_psum.tile([128, 1024], F32, tag="qk", name=f"qk{ki % 2}")
            nc.tensor.matmul(
                qk[:kwid, 0:qn], kT[c6][0:64, ksl], qT[c6][0:64, q0:q0 + qn],
                start=True, stop=True, tile_position=(0, 0),
            )
            nc.tensor.matmul(
                qk[:kwid, 512:512 + qn], kT[c6][64:128, ksl],
                qT[c6][64:128, q0:q0 + qn],
                start=True, stop=True, tile_position=(64, 0),
            )
            return qk

        # ---------------- main attention pipeline
        with nc.named_scope("attn"):
            if V_SERIAL:
                for k in range(KT):
                    emit_vproj(k)
            for qbi in range(NQB):
                q0 = qbi * QB
                for c6 in range(NCH):
                    # required projections for THIS (qbi, c6), in PE order
                    if V_SERIAL:
                        filler_emit(1)
                    if qbi == 0:
                        for (a, b) in QSL:
                            emit_proj(kw, tT, kT, kb_t, c6, a, b)
                        for (a, b) in (QSL if V_QALL else [(0, QB)]):
                            emit_proj(qw, sT, qT, qb_t, c6, a, b)
                        if c6 == 0 and not V_SERIAL:
                            emit_vproj(0)
                            emit_vproj(1)
                    else:
                        if not V_QALL:
                            if qbi == 1:
                                emit_proj(qw, sT, qT, qb_t, c6, QB, QB)
                            if qbi == 2:
                                emit_proj(qw, sT, qT, qb_t, c6, 2 * QB, QB)
                        filler.append(
                            (lambda c=c6, a=(qbi - 1) * QB:
                             emit_outproj(c, a, QB)))

                    avA = av_psum.tile([128, 512], F32, tag="av", name="avA")
                    avB = av_psum.tile([128, 512], F32, tag="av", name="avB")

                    qk_next = qk_mm(c6, 0, q0, QB)
                    for ki in range(KT):
                        kwid = kw_of(ki)
                        qk = qk_next
                        if V_SERIAL:
                            pass
                        elif qbi == 0 and c6 == 0 and ki + 2 < KT:
                            emit_vproj(ki + 2)
                        elif filler and (ki % 4 == 3):
                            filler_emit(1)
                        if ki + 1 < KT:
                            qk_next = qk_mm(c6, ki + 1, q0, QB)
                        ex = P_pool.tile([128, 2, 512], BF16, tag="P")
                        nc.scalar.activation(
                            ex[:kwid, :, :],
                            qk[:kwid, :].rearrange("p (h q) -> p h q", h=2),
                            mybir.ActivationFunctionType.Exp,
                        )
                        if DBG and qbi == 0 and c6 == 0 and ki == 0:
                            nc.sync.dma_start(dbg_P[:], ex[:, :, :])
                        for hh in range(2):
                            nc.tensor.matmul(
                                (avA if hh == 0 else avB)[:, 0:QB],
                                vP[ki][:kwid, hh, c6, :],
                                ex[:kwid, hh, :],
                                start=(ki == 0), stop=(ki == KT - 1),
                            )
                    av_evac(c6, avA, avB, q0, QB)

            # ---- rump q block (32 cols): batch QK psum across all ki
            q0 = NQB * QB
            with nc.named_scope("rump"):
                for c6 in (range(NCH) if V_RUMP else []):
                    if not V_QALL:
                        emit_proj(qw, sT, qT, qb_t, c6, q0, QR)
                    avA = av_psum.tile([128, 512], F32, tag="av", name="avA")
                    avB = av_psum.tile([128, 512], F32, tag="av", name="avB")
                    qk = qk_psum.tile([128, 1024], F32, tag="qk", name="qkr")
                    # head-major, ki padded to 16: head A fills psum bank 0,
                    # head B bank 1 (concurrent pair must hit distinct banks)
                    qkv = qk[:, :].rearrange("p (h k q) -> p h k q", h=2, k=16)
                    for ki in range(KT):
                        kwid = kw_of(ki)
                        ksl = slice(ki * 128, ki * 128 + kwid)
                        nc.tensor.matmul(
                            qkv[:kwid, 0, ki, :], kT[c6][0:64, ksl],
                            qT[c6][0:64, q0:q0 + QR],
                            start=True, stop=True, tile_position=(0, 0),
                        )
                        nc.tensor.matmul(
                            qkv[:kwid, 1, ki, :], kT[c6][64:128, ksl],
                            qT[c6][64:128, q0:q0 + QR],
                            start=True, stop=True, tile_position=(64, 0),
                        )
                    ex = P_pool.tile([128, 2, KT, QR], BF16, tag="Pr")
                    for hh in range(2):
                        nc.scalar.activation(
                            ex[:, hh, 0:12, :], qkv[:, hh, 0:12, :],
                            mybir.ActivationFunctionType.Exp,
                        )
                        nc.scalar.activation(
                            ex[0:KR, hh, 12, :], qkv[0:KR, hh, 12, :],
                            mybir.ActivationFunctionType.Exp,
                        )
                    filler_emit(1)
                    for ki in range(KT):
                        kwid = kw_of(ki)
                        for hh in range(2):
                            nc.tensor.matmul(
                                (avA if hh == 0 else avB)[:, 0:QR],
                                vP[ki][:kwid, hh, c6, :],
                                ex[:kwid, hh, ki, :],
                                start=(ki == 0), stop=(ki == KT - 1),
                            )
                    av_evac(c6, avA, avB, q0, QR)
                    filler.append(
                        (lambda c=c6, a=(NQB - 1) * QB:
                         emit_outproj(c, a, QB)))

            if DBG:
                nc.sync.dma_start(dbg_sT[:], sT[0][:])
                for c in range(NCH):
                    nc.sync.dma_start(dbg_qT[c], qT[c][:])
                nc.sync.dma_start(dbg_kT[:], kT[0][:])
                nc.sync.dma_start(dbg_vP[:], vP[0][:])
                for c in range(NCH):
                    nc.sync.dma_start(dbg_OT[c], OT[c][:])
            # ---- drain remaining filler + final out-proj slices
            with nc.named_scope("tail"):
                filler_emit(len(filler))
                if V_RUMP:
                    for c6 in range(NCH):
                        emit_outproj(c6, NQB * QB, QR)

    nc.finalize()
    return nc


def _install_axon_ntff_shim():
    if "antenv.axon_hooks" in sys.modules:
        return
    mod = types.ModuleType("antenv.axon_hooks")
    mod._hook = None
    mod.set_axon_ntff_profile_hook = lambda h: setattr(mod, "_hook", h)
    mod.get_axon_ntff_profile_hook = lambda: mod._hook
    sys.modules["antenv.axon_hooks"] = mod
    try:
        import antenv

        antenv.axon_hooks = mod
        from trn_agent_boot.trn_boot import _ntff_profile_via_ctypes

        hook = _ntff_profile_via_ctypes("/opt/axon/libaxon_pjrt.so")
        if hook is not None:
            mod.set_axon_ntff_profile_hook(hook)
    except Exception:
        pass


def prep_inputs(s_x, t_x, clip_space_pos, vmae_space_pos, clip_temporal_pos,
                vmae_temporal_pos, q_w, q_b, kv_w, kv_b, proj_w, proj_b):
    """Host-side sharding/layout prep. Returns list of 8 per-core input maps."""
    f = np.float32
    bf = ml_dtypes.bfloat16
    pos_sT = np.ascontiguousarray(
        (clip_space_pos.T[:, :, None] + clip_temporal_pos.T[:, None, :])
        .reshape(DIM, NT)).astype(bf)
    pos_tT = np.ascontiguousarray(
        (vmae_space_pos.T[:, :, None] + vmae_temporal_pos.T[:, None, :])
        .reshape(DIM, NT)).astype(bf)
    q_wT = np.ascontiguousarray(np.asarray(q_w).T * SCALE).astype(bf)
    k_wT = np.ascontiguousarray(np.asarray(kv_w)[:DIM].T).astype(bf)
    # v weight rows reordered: [even heads' v dims | odd heads' v dims]
    v_w = np.asarray(kv_w)[DIM:]         # [768 out, 768 in]
    v_b = np.asarray(kv_b)[DIM:]
    order = np.concatenate([
        np.arange(DIM).reshape(H, DH)[0::2].reshape(-1),
        np.arange(DIM).reshape(H, DH)[1::2].reshape(-1),
    ])
    v_wT = np.ascontiguousarray(v_w[order].T).astype(bf)
    v_br = np.ascontiguousarray(
        np.broadcast_to(v_b[order].reshape(1, DIM), (128, DIM)), dtype=f)
    proj_wT = np.ascontiguousarray(np.asarray(proj_w).T).astype(bf)
    q_b2 = np.ascontiguousarray(
        (np.asarray(q_b) * SCALE).reshape(NCH, 128).T, dtype=f)
    k_b2 = np.ascontiguousarray(
        np.asarray(kv_b)[:DIM].reshape(NCH, 128).T, dtype=f)
    p_b2 = np.ascontiguousarray(np.asarray(proj_b).reshape(NCH, 128).T, dtype=f)

    in_maps = []
    for b in range(B):
        s_slice = np.asarray(s_x)[:, b * TS:(b + 1) * TS, :]  # (196, 8, 768)
        t_slice = np.asarray(t_x)[1:, b * T:(b + 1) * T, :]   # (196, 8, 768)
        s_xT = np.ascontiguousarray(
            s_slice.transpose(2, 0, 1).reshape(DIM, NT)).astype(bf)
        t_xT = np.ascontiguousarray(
            t_slice.transpose(2, 0, 1).reshape(DIM, NT)).astype(bf)
        in_maps.append({
            "s_xT": s_xT, "t_xT": t_xT,
            "pos_sT": pos_sT, "pos_tT": pos_tT,
            "q_wT": q_wT, "k_wT": k_wT, "v_wT": v_wT, "proj_wT": proj_wT,
            "q_b2": q_b2, "k_b2": k_b2, "p_b2": p_b2, "v_br": v_br,
        })
    return in_maps


def unshard_output(results):
    """results: list of 8 dicts with 'outT' [768, 1568] -> (196, 64, 768)."""
    out = np.empty((APATCH, B * TS, DIM), dtype=np.float32)
    for b in range(B):
        o = results[b]["outT"].reshape(DIM, APATCH, TS)
        out[:, b * TS:(b + 1) * TS, :] = o.transpose(1, 2, 0)
    return out


def kernel(**inputs):
    _install_axon_ntff_shim()
    in_maps = prep_inputs(**inputs)
    if "nc" not in _NC_CACHE:
        _NC_CACHE["nc"] = build_nc()
    nc = _NC_CACHE["nc"]
    res = run_bass_kernel_spmd(nc, in_maps, core_ids=list(range(B)))
    return unshard_output(res.results)


if __name__ == "__main__":
    rng = np.random.default_rng(0)
    fake = {
        "s_x": rng.standard_normal((APATCH, B * TS, DIM), dtype=np.float32),
        "t_x": rng.standard_normal((VP + 1, B * T, DIM), dtype=np.float32),
        "clip_space_pos": SCALE * rng.standard_normal((APATCH, DIM), dtype=np.float32),
        "vmae_space_pos": SCALE * rng.standard_normal((VP, DIM), dtype=np.float32),
        "clip_temporal_pos": SCALE * rng.standard_normal((TS, DIM), dtype=np.float32),
        "vmae_temporal_pos": SCALE * rng.standard_normal((T, DIM), dtype=np.float32),
        "q_w": (0.02 * rng.standard_normal((DIM, DIM))).astype(np.float32),
        "q_b": np.zeros(DIM, np.float32),
        "kv_w": (0.02 * rng.standard_normal((2 * DIM, DIM))).astype(np.float32),
        "kv_b": np.zeros(2 * DIM, np.float32),
        "proj_w": (0.02 * rng.standard_normal((DIM, DIM))).astype(np.float32),
        "proj_b": np.zeros(DIM, np.float32),
    }
    out = kernel(**fake)
    print("out", out.shape, out.dtype)


# revision 16
# speedup vs baseline: 1.4533x; 1.0130x over previous
"""Trainium2 Bass kernel for nn_CrossAttentionT2S (fused pos-embed cross-attention).

Sharding: data-parallel over the true batch axis b=8, one batch element per
NeuronCore. All tensors bf16 on device; feature-major ("transposed",
[feature, token]) layouts so matmuls contract over the partition dim.

Per core (NT=1568 q tokens, 1568 kv tokens, 12 heads, dh=64):
  tT = t_xT + pos_tT ; sT = s_xT + pos_sT          (DVE bf16 adds)
  kT = k_w @ t (feature-major, 6 chunks of 2 heads) (PE, evac on DVE + bias)
  qT = (q_w*SCALE) @ s + q_b*SCALE                  (PE, evac DVE)
  V' = token-major [128tok, parity, 6, 128]: even heads [v|ones64],
       odd heads [ones64|v]                          (PE, evac DVE, ones memset)
  per (qb in 512,512,512 + rump32, c6 head-pair, ki in 13):
    S[k128, q512]x2 heads — two row-tiled matmuls (0,0)/(64,0), concurrent
    P = exp(S) — ONE ScalarE activation per (c6,qb,ki), [128, 1024] free
    O~/den: AV matmul lhsT=V'[h] M=128: 64 cols of v + 64 ones columns ->
       psum [128,512]: O~ on one 64-partition half, den replicated on other
    evac: DVE reciprocal_approx_fast(den half) -> rcp, DVE mul -> OT bf16
  out = proj_w @ O + proj_b (PE, interleaved with next qb; DVE evac, DMA out)

ScalarE runs ONLY exp (the structural bottleneck ~275us); everything else is
kept off it. Projections/out-proj are emitted interleaved with attention so
the PE fills its exp-wait gaps and ACT never idles after warmup.
"""
import sys
import types
from contextlib import ExitStack

import numpy as np
import ml_dtypes

import concourse.bass as bass
import concourse.mybir as mybir
import concourse.tile as tile
from concourse import bacc
from concourse.bass_utils import run_bass_kernel_spmd

# ---------------------------------------------------------------- constants
DIM = 768
H = 12
DH = 64
T = 8
TS = 8
APATCH = 196
VP = 196
B = 8
NT = APATCH * TS          # 1568 tokens per core, both q and kv side
SCALE = DH ** -0.5
NCH = DIM // 128          # 6 feature chunks (2 heads each)
KT = 13                   # k tiles: 12 full 128 + rump 32
KR = NT - 12 * 128        # 32
QB = 512                  # q block
NQB = 3                   # full q blocks; rump = 32
QR = NT - NQB * QB        # 32
F32 = mybir.dt.float32
BF16 = mybir.dt.bfloat16
ADD = mybir.AluOpType.add
MULT = mybir.AluOpType.mult

_NC_CACHE = {}

import os
V_RECIP = os.environ.get("KV_RECIP", "fast")     # fast | exact
V_MEMSET = os.environ.get("KV_MEMSET", "pool")   # pool | dve
V_RUMP = os.environ.get("KV_RUMP", "1") == "1"
V_QALL = os.environ.get("KV_QALL", "0") == "1"
V_SERIAL = os.environ.get("KV_SERIAL", "0") == "1"



def kw_of(ki):
    return 128 if ki < 12 else KR


def build_nc():
    nc = bacc.Bacc(None)

    s_xT = nc.dram_tensor("s_xT", [DIM, NT], BF16, kind="ExternalInput")
    t_xT = nc.dram_tensor("t_xT", [DIM, NT], BF16, kind="ExternalInput")
    pos_sT = nc.dram_tensor("pos_sT", [DIM, NT], BF16, kind="ExternalInput")
    pos_tT = nc.dram_tensor("pos_tT", [DIM, NT], BF16, kind="ExternalInput")
    q_wT = nc.dram_tensor("q_wT", [DIM, DIM], BF16, kind="ExternalInput")
    k_wT = nc.dram_tensor("k_wT", [DIM, DIM], BF16, kind="ExternalInput")
    v_wT = nc.dram_tensor("v_wT", [DIM, DIM], BF16, kind="ExternalInput")
    proj_wT = nc.dram_tensor("proj_wT", [DIM, DIM], BF16, kind="ExternalInput")
    q_b2 = nc.dram_tensor("q_b2", [128, NCH], F32, kind="ExternalInput")
    k_b2 = nc.dram_tensor("k_b2", [128, NCH], F32, kind="ExternalInput")
    p_b2 = nc.dram_tensor("p_b2", [128, NCH], F32, kind="ExternalInput")
    v_br = nc.dram_tensor("v_br", [128, DIM], F32, kind="ExternalInput")
    outT = nc.dram_tensor("outT", [DIM, NT], F32, kind="ExternalOutput")
    DBG = os.environ.get("KV_DBG", "0") == "1"
    if DBG:
        dbg_qT = nc.dram_tensor("dbg_qT", [NCH, 128, NT], BF16, kind="ExternalOutput")
        dbg_kT = nc.dram_tensor("dbg_kT", [128, NT], BF16, kind="ExternalOutput")
        dbg_vP = nc.dram_tensor("dbg_vP", [128, 2, NCH, 128], BF16, kind="ExternalOutput")
        dbg_OT = nc.dram_tensor("dbg_OT", [NCH, 128, NT], BF16, kind="ExternalOutput")
        dbg_P = nc.dram_tensor("dbg_P", [128, 2, 512], BF16, kind="ExternalOutput")
        dbg_sT = nc.dram_tensor("dbg_sT", [128, NT], BF16, kind="ExternalOutput")

    with tile.TileContext(nc) as tc, ExitStack() as top:
        # ---------------- constant / persistent tiles
        cpool = top.enter_context(tc.tile_pool(name="consts", bufs=1))
        qb_t = cpool.tile([128, NCH], F32, tag="qb")
        kb_t = cpool.tile([128, NCH], F32, tag="kb")
        pb_t = cpool.tile([128, NCH], F32, tag="pb")
        vb_t = cpool.tile([128, DIM], F32, tag="vb")
        nc.sync.dma_start(qb_t[:], q_b2[:])
        nc.sync.dma_start(kb_t[:], k_b2[:])
        nc.sync.dma_start(pb_t[:], p_b2[:])
        nc.sync.dma_start(vb_t[:], v_br[:])

        w_pool = top.enter_context(tc.tile_pool(name="w", bufs=NCH))
        qw = [w_pool.tile([128, DIM], BF16, tag="qw", name=f"qw{c}") for c in range(NCH)]
        kw = [w_pool.tile([128, DIM], BF16, tag="kw", name=f"kw{c}") for c in range(NCH)]
        vw = [w_pool.tile([128, DIM], BF16, tag="vw", name=f"vw{c}") for c in range(NCH)]
        pw = [w_pool.tile([128, DIM], BF16, tag="pw", name=f"pw{c}") for c in range(NCH)]
        for c in range(NCH):
            sl = slice(c * 128, (c + 1) * 128)
            nc.gpsimd.dma_start(kw[c][:], k_wT[sl, :])
        for c in range(NCH):
            sl = slice(c * 128, (c + 1) * 128)
            nc.gpsimd.dma_start(vw[c][:], v_wT[sl, :])
            nc.sync.dma_start(qw[c][:], q_wT[sl, :])
            nc.sync.dma_start(pw[c][:], proj_wT[sl, :])

        # x + pos, bf16 feature-major
        xs_pool = top.enter_context(tc.tile_pool(name="xs", bufs=NCH))
        sT = [xs_pool.tile([128, NT], BF16, tag="sT", name=f"sT{c}") for c in range(NCH)]
        tT = [xs_pool.tile([128, NT], BF16, tag="tT", name=f"tT{c}") for c in range(NCH)]
        with ExitStack() as pr, nc.named_scope("p0_load"):
            xin_pool = pr.enter_context(tc.tile_pool(name="xin", bufs=4))
            pos_pool = pr.enter_context(tc.tile_pool(name="pos", bufs=4))
            xts, pts = [], []
            for c in range(NCH):
                sl = slice(c * 128, (c + 1) * 128)
                xt = xin_pool.tile([128, NT], BF16, tag="xin", name=f"xt{c}")
                pt = pos_pool.tile([128, NT], BF16, tag="pos", name=f"pt{c}")
                nc.gpsimd.dma_start(xt[:], t_xT[sl, :])
                nc.scalar.dma_start(pt[:], pos_tT[sl, :])
                nc.vector.tensor_add(tT[c][:], xt[:], pt[:])
            for c in range(NCH):
                sl = slice(c * 128, (c + 1) * 128)
                xs2 = xin_pool.tile([128, NT], BF16, tag="xin", name=f"xs{c}")
                ps2 = pos_pool.tile([128, NT], BF16, tag="pos", name=f"ps{c}")
                nc.sync.dma_start(xs2[:], s_xT[sl, :])
                nc.scalar.dma_start(ps2[:], pos_sT[sl, :])
                nc.vector.tensor_add(sT[c][:], xs2[:], ps2[:])

        # q/k feature-major bf16; V' token-major bf16 with ones blocks
        qkT_pool = top.enter_context(tc.tile_pool(name="qkT", bufs=NCH))
        qT = [qkT_pool.tile([128, NT], BF16, tag="qT", name=f"qT{c}") for c in range(NCH)]
        kT = [qkT_pool.tile([128, NT], BF16, tag="kT", name=f"kT{c}") for c in range(NCH)]
        vP_pool = top.enter_context(tc.tile_pool(name="vP", bufs=KT))
        # layout: [tok, parity, pair, 128]; head h = 2*pair+parity
        vP = [vP_pool.tile([128, 2, NCH, 128], BF16, tag="vP", name=f"vP{k}")
              for k in range(KT)]
        ms_eng = nc.gpsimd if V_MEMSET == "pool" else nc.vector
        for k in range(KT):
            kwid = kw_of(k)
            ms_eng.memset(vP[k][:kwid, :, :, 0:64], 1.0)

        OT_pool = top.enter_context(tc.tile_pool(name="OT", bufs=NCH))
        OT = [OT_pool.tile([128, NT], BF16, tag="OT", name=f"OT{c}") for c in range(NCH)]

        # ---------------- psum pools (8 banks total)
        qk_psum = top.enter_context(tc.tile_pool(name="qkps", bufs=2, space="PSUM"))
        av_psum = top.enter_context(tc.tile_pool(name="avps", bufs=2, space="PSUM"))
        gm_psum = top.enter_context(tc.tile_pool(name="gmps", bufs=2, space="PSUM"))

        P_pool = top.enter_context(tc.tile_pool(name="P", bufs=3))
        rcp_pool = top.enter_context(tc.tile_pool(name="rcp", bufs=2))
        ost_pool = top.enter_context(tc.tile_pool(name="ost", bufs=2))

        QSL = [(i * QB, QB) for i in range(NQB)] + [(NQB * QB, QR)]

        def emit_proj(ws, xsrc, dst, bias_t, c_out, q0, qn):
            """dst[c_out][:, q0:q0+qn] = ws.T @ x (+bias), bf16 evac on DVE."""
            ps = gm_psum.tile([128, 512], F32, tag="gm")
            for c in range(NCH):
                nc.tensor.matmul(
                    ps[:, 0:qn],
                    ws[c][:, c_out * 128:(c_out + 1) * 128],
                    xsrc[c][:, q0:q0 + qn],
                    start=(c == 0), stop=(c == NCH - 1),
                )
            nc.vector.tensor_scalar_add(
                dst[c_out][:, q0:q0 + qn], ps[:, 0:qn], bias_t[:, c_out:c_out + 1]
            )

        def emit_vproj(k):
            """V' for k-tile k. v_wT cols pre-reordered on host:
            group0 = even heads' v dims, group1 = odd heads'."""
            kwid = kw_of(k)
            for g in range(2):
                ps = gm_psum.tile([128, 512], F32, tag="gm")
                for c in range(NCH):
                    nc.tensor.matmul(
                        ps[:kwid, 0:384],
                        tT[c][:, k * 128:k * 128 + kwid],
                        vw[c][:, g * 384:(g + 1) * 384],
                        start=(c == 0), stop=(c == NCH - 1),
                    )
                dst = vP[k][:kwid, g, :, 64:128]
                src = ps[:kwid, 0:384].rearrange("p (h d) -> p h d", d=DH)
                bia = vb_t[:kwid, g * 384:(g + 1) * 384].rearrange(
                    "p (h d) -> p h d", d=DH)
                nc.vector.tensor_tensor(dst, src, bia, ADD)

        def emit_outproj(c_out, q0, qn):
            ps = gm_psum.tile([128, 512], F32, tag="gm")
            for c in range(NCH):
                nc.tensor.matmul(
                    ps[:, 0:qn],
                    pw[c][:, c_out * 128:(c_out + 1) * 128],
                    OT[c][:, q0:q0 + qn],
                    start=(c == 0), stop=(c == NCH - 1),
                )
            oe = ost_pool.tile([128, 512], F32, tag="ost")
            nc.vector.tensor_scalar_add(
                oe[:, 0:qn], ps[:, 0:qn], pb_t[:, c_out:c_out + 1]
            )
            nc.sync.dma_start(outT[c_out * 128:(c_out + 1) * 128, q0:q0 + qn],
                              oe[:, 0:qn])

        filler = []  # deferred out-proj emissions (no forward PE deps)

        def filler_emit(n):
            for _ in range(n):
                if filler:
                    filler.pop(0)()

        def av_evac(c6, avA, avB, q0, qn):
            """Normalize + evac both heads of chunk c6 for q slice [q0, q0+qn)."""
            # both heads: den replicated at psum parts 0:64 (base-0 for the
            # custom DVE recip), O~ at 64:128; rcp written at base-0 SBUF.
            rcp = rcp_pool.tile([128, 1024], F32, tag="rcp")
            recip = (nc.vector.reciprocal_approx_fast if V_RECIP == "fast"
                     else nc.vector.reciprocal)
            recip(rcp[0:64, 0:qn], avA[0:64, 0:qn])
            recip(rcp[0:64, 512:512 + qn], avB[0:64, 0:qn])
            nc.vector.tensor_tensor(
                OT[c6][0:64, q0:q0 + qn], avA[64:128, 0:qn], rcp[0:64, 0:qn],
                MULT)
            nc.vector.tensor_tensor(
                OT[c6][64:128, q0:q0 + qn], avB[64:128, 0:qn],
                rcp[0:64, 512:512 + qn], MULT)

        def qk_mm(c6, ki, q0, qn):
            kwid = kw_of(ki)
            ksl = slice(ki * 128, ki * 128 + kwid)
            qk = qk_psum.tile([128, 1024], F32, tag="qk", name=f"qk{ki % 2}")
            nc.tensor.matmul(
                qk[:kwid, 0:qn], kT[c6][0:64, ksl], qT[c6][0:64, q0:q0 + qn],
                start=True, stop=True, tile_position=(0, 0),
            )
            nc.tensor.matmul(
                qk[:kwid, 512:512 + qn], kT[c6][64:128, ksl],
                qT[c6][64:128, q0:q0 + qn],
                start=True, stop=True, tile_position=(64, 0),
            )
            return qk

        # ---------------- main attention pipeline
        with nc.named_scope("attn"):
            if V_SERIAL:
                for k in range(KT):
                    emit_vproj(k)
            for qbi in range(NQB):
                q0 = qbi * QB
                for c6 in range(NCH):
                    # required projections for THIS (qbi, c6), in PE order
                    if V_SERIAL:
                        filler_emit(1)
                    if qbi == 0:
                        if c6 == 0 and not V_SERIAL:
                            # critical path: only what QK(ki=0..3) needs
                            emit_proj(kw, tT, kT, kb_t, 0, 0, QB)
                            emit_proj(qw, sT, qT, qb_t, 0, 0, QB)
                            emit_vproj(0)
                            emit_vproj(1)
                        else:
                            for (a, b) in QSL:
                                emit_proj(kw, tT, kT, kb_t, c6, a, b)
                            for (a, b) in (QSL if V_QALL else [(0, QB)]):
                                emit_proj(qw, sT, qT, qb_t, c6, a, b)
                    else:
                        if not V_QALL:
                            if qbi == 1:
                                emit_proj(qw, sT, qT, qb_t, c6, QB, QB)
                            if qbi == 2:
                                emit_proj(qw, sT, qT, qb_t, c6, 2 * QB, QB)
                        filler.append(
                            (lambda c=c6, a=(qbi - 1) * QB:
                             emit_outproj(c, a, QB)))

                    avA = av_psum.tile([128, 512], F32, tag="av", name="avA")
                    avB = av_psum.tile([128, 512], F32, tag="av", name="avB")

                    qk_next = qk_mm(c6, 0, q0, QB)
                    for ki in range(KT):
                        kwid = kw_of(ki)
                        qk = qk_next
                        if V_SERIAL:
                            pass
                        elif qbi == 0 and c6 == 0:
                            if ki + 2 < KT:
                                emit_vproj(ki + 2)
                            if ki in (1, 4, 7):
                                a = (ki + 2) // 3 * QB
                                emit_proj(kw, tT, kT, kb_t, 0, a,
                                          QB if a < NQB * QB else QR)
                        elif filler and (ki % 4 == 3):
                            filler_emit(1)
                        if ki + 1 < KT:
                            qk_next = qk_mm(c6, ki + 1, q0, QB)
                        ex = P_pool.tile([128, 2, 512], BF16, tag="P")
                        nc.scalar.activation(
                            ex[:kwid, :, :],
                            qk[:kwid, :].rearrange("p (h q) -> p h q", h=2),
                            mybir.ActivationFunctionType.Exp,
                        )
                        if DBG and qbi == 0 and c6 == 0 and ki == 0:
                            nc.sync.dma_start(dbg_P[:], ex[:, :, :])
                        for hh in range(2):
                            nc.tensor.matmul(
                                (avA if hh == 0 else avB)[:, 0:QB],
                                vP[ki][:kwid, hh, c6, :],
                                ex[:kwid, hh, :],
                                start=(ki == 0), stop=(ki == KT - 1),
                            )
                    av_evac(c6, avA, avB, q0, QB)

            # ---- rump q block (32 cols): batch QK psum across all ki
            q0 = NQB * QB
            with nc.named_scope("rump"):
                for c6 in (range(NCH) if V_RUMP else []):
                    if not V_QALL:
                        emit_proj(qw, sT, qT, qb_t, c6, q0, QR)
                    avA = av_psum.tile([128, 512], F32, tag="av", name="avA")
                    avB = av_psum.tile([128, 512], F32, tag="av", name="avB")
                    qk = qk_psum.tile([128, 1024], F32, tag="qk", name="qkr")
                    # head-major, ki padded to 16: head A fills psum bank 0,
                    # head B bank 1 (concurrent pair must hit distinct banks)
                    qkv = qk[:, :].rearrange("p (h k q) -> p h k q", h=2, k=16)
                    for ki in range(KT):
                        kwid = kw_of(ki)
                        ksl = slice(ki * 128, ki * 128 + kwid)
                        nc.tensor.matmul(
                            qkv[:kwid, 0, ki, :], kT[c6][0:64, ksl],
                            qT[c6][0:64, q0:q0 + QR],
                            start=True, stop=True, tile_position=(0, 0),
                        )
                        nc.tensor.matmul(
                            qkv[:kwid, 1, ki, :], kT[c6][64:128, ksl],
                            qT[c6][64:128, q0:q0 + QR],
                            start=True, stop=True, tile_position=(64, 0),
                        )
                    ex = P_pool.tile([128, 2, KT, QR], BF16, tag="Pr")
                    for hh in range(2):
                        nc.scalar.activation(
                            ex[:, hh, 0:12, :], qkv[:, hh, 0:12, :],
                            mybir.ActivationFunctionType.Exp,
                        )
                        nc.scalar.activation(
                            ex[0:KR, hh, 12, :], qkv[0:KR, hh, 12, :],
                            mybir.ActivationFunctionType.Exp,
                        )
                    filler_emit(2)
                    for ki in range(KT):
                        kwid = kw_of(ki)
                        for hh in range(2):
                            nc.tensor.matmul(
                                (avA if hh == 0 else avB)[:, 0:QR],
                                vP[ki][:kwid, hh, c6, :],
                                ex[:kwid, hh, ki, :],
                                start=(ki == 0), stop=(ki == KT - 1),
                            )
                    av_evac(c6, avA, avB, q0, QR)
                    filler.append(
                        (lambda c=c6, a=(NQB - 1) * QB:
                         emit_outproj(c, a, QB)))

            if DBG:
                nc.sync.dma_start(dbg_sT[:], sT[0][:])
                for c in range(NCH):
                    nc.sync.dma_start(dbg_qT[c], qT[c][:])
                nc.sync.dma_start(dbg_kT[:], kT[0][:])
                nc.sync.dma_start(dbg_vP[:], vP[0][:])
                for c in range(NCH):
                    nc.sync.dma_start(dbg_OT[c], OT[c][:])
            # ---- drain remaining filler + final out-proj slices
            with nc.named_scope("tail"):
                filler_emit(len(filler))
                if V_RUMP:
                    for c6 in range(NCH):
                        emit_outproj(c6, NQB * QB, QR)

    nc.finalize()
    return nc


def _install_axon_ntff_shim():
    if "antenv.axon_hooks" in sys.modules:
        return
    mod = types.ModuleType("antenv.axon_hooks")
    mod._hook = None
    mod.set_axon_ntff_profile_hook = lambda h: setattr(mod, "_hook", h)
    mod.get_axon_ntff_profile_hook = lambda: mod._hook
    sys.modules["antenv.axon_hooks"] = mod
    try:
        import antenv

        antenv.axon_hooks = mod
        from trn_agent_boot.trn_boot import _ntff_profile_via_ctypes

        hook = _ntff_profile_via_ctypes("/opt/axon/libaxon_pjrt.so")
        if hook is not None:
            mod.set_axon_ntff_profile_hook(hook)
    except Exception:
        pass


def prep_inputs(s_x, t_x, clip_space_pos, vmae_space_pos, clip_temporal_pos,
                vmae_temporal_pos, q_w, q_b, kv_w, kv_b, proj_w, proj_b):
    """Host-side sharding/layout prep. Returns list of 8 per-core input maps."""
    f = np.float32
    bf = ml_dtypes.bfloat16
    pos_sT = np.ascontiguousarray(
        (clip_space_pos.T[:, :, None] + clip_temporal_pos.T[:, None, :])
        .reshape(DIM, NT)).astype(bf)
    pos_tT = np.ascontiguousarray(
        (vmae_space_pos.T[:, :, None] + vmae_temporal_pos.T[:, None, :])
        .reshape(DIM, NT)).astype(bf)
    q_wT = np.ascontiguousarray(np.asarray(q_w).T * SCALE).astype(bf)
    k_wT = np.ascontiguousarray(np.asarray(kv_w)[:DIM].T).astype(bf)
    # v weight rows reordered: [even heads' v dims | odd heads' v dims]
    v_w = np.asarray(kv_w)[DIM:]         # [768 out, 768 in]
    v_b = np.asarray(kv_b)[DIM:]
    order = np.concatenate([
        np.arange(DIM).reshape(H, DH)[0::2].reshape(-1),
        np.arange(DIM).reshape(H, DH)[1::2].reshape(-1),
    ])
    v_wT = np.ascontiguousarray(v_w[order].T).astype(bf)
    v_br = np.ascontiguousarray(
        np.broadcast_to(v_b[order].reshape(1, DIM), (128, DIM)), dtype=f)
    proj_wT = np.ascontiguousarray(np.asarray(proj_w).T).astype(bf)
    q_b2 = np.ascontiguousarray(
        (np.asarray(q_b) * SCALE).reshape(NCH, 128).T, dtype=f)
    k_b2 = np.ascontiguousarray(
        np.asarray(kv_b)[:DIM].reshape(NCH, 128).T, dtype=f)
    p_b2 = np.ascontiguousarray(np.asarray(proj_b).reshape(NCH, 128).T, dtype=f)

    in_maps = []
    for b in range(B):
        s_slice = np.asarray(s_x)[:, b * TS:(b + 1) * TS, :]  # (196, 8, 768)
        t_slice = np.asarray(t_x)[1:, b * T:(b + 1) * T, :]   # (196, 8, 768)
        s_xT = np.ascontiguousarray(
            s_slice.transpose(2, 0, 1).reshape(DIM, NT)).astype(bf)
        t_xT = np.ascontiguousarray(
            t_slice.transpose(2, 0, 1).reshape(DIM, NT)).astype(bf)
        in_maps.append({
            "s_xT": s_xT, "t_xT": t_xT,
            "pos_sT": pos_sT, "pos_tT": pos_tT,
            "q_wT": q_wT, "k_wT": k_wT, "v_wT": v_wT, "proj_wT": proj_wT,
            "q_b2": q_b2, "k_b2": k_b2, "p_b2": p_b2, "v_br": v_br,
        })
    return in_maps


def unshard_output(results):
    """results: list of 8 dicts with 'outT' [768, 1568] -> (196, 64, 768)."""
    out = np.empty((APATCH, B * TS, DIM), dtype=np.float32)
    for b in range(B):
        o = results[b]["outT"].reshape(DIM, APATCH, TS)
        out[:, b * TS:(b + 1) * TS, :] = o.transpose(1, 2, 0)
    return out


def kernel(**inputs):
    _install_axon_ntff_shim()
    in_maps = prep_inputs(**inputs)
    if "nc" not in _NC_CACHE:
        _NC_CACHE["nc"] = build_nc()
    nc = _NC_CACHE["nc"]
    res = run_bass_kernel_spmd(nc, in_maps, core_ids=list(range(B)))
    return unshard_output(res.results)


if __name__ == "__main__":
    rng = np.random.default_rng(0)
    fake = {
        "s_x": rng.standard_normal((APATCH, B * TS, DIM), dtype=np.float32),
        "t_x": rng.standard_normal((VP + 1, B * T, DIM), dtype=np.float32),
        "clip_space_pos": SCALE * rng.standard_normal((APATCH, DIM), dtype=np.float32),
        "vmae_space_pos": SCALE * rng.standard_normal((VP, DIM), dtype=np.float32),
        "clip_temporal_pos": SCALE * rng.standard_normal((TS, DIM), dtype=np.float32),
        "vmae_temporal_pos": SCALE * rng.standard_normal((T, DIM), dtype=np.float32),
        "q_w": (0.02 * rng.standard_normal((DIM, DIM))).astype(np.float32),
        "q_b": np.zeros(DIM, np.float32),
        "kv_w": (0.02 * rng.standard_normal((2 * DIM, DIM))).astype(np.float32),
        "kv_b": np.zeros(2 * DIM, np.float32),
        "proj_w": (0.02 * rng.standard_normal((DIM, DIM))).astype(np.float32),
        "proj_b": np.zeros(DIM, np.float32),
    }
    out = kernel(**fake)
    print("out", out.shape, out.dtype)


# revision 17
# speedup vs baseline: 1.4541x; 1.0006x over previous
"""Trainium2 Bass kernel for nn_CrossAttentionT2S (fused pos-embed cross-attention).

Sharding: data-parallel over the true batch axis b=8, one batch element per
NeuronCore. All tensors bf16 on device; feature-major ("transposed",
[feature, token]) layouts so matmuls contract over the partition dim.

Per core (NT=1568 q tokens, 1568 kv tokens, 12 heads, dh=64):
  tT = t_xT + pos_tT ; sT = s_xT + pos_sT          (DVE bf16 adds)
  kT = k_w @ t (feature-major, 6 chunks of 2 heads) (PE, evac on DVE + bias)
  qT = (q_w*SCALE) @ s + q_b*SCALE                  (PE, evac DVE)
  V' = token-major [128tok, parity, 6, 128]: even heads [v|ones64],
       odd heads [ones64|v]                          (PE, evac DVE, ones memset)
  per (qb in 512,512,512 + rump32, c6 head-pair, ki in 13):
    S[k128, q512]x2 heads — two row-tiled matmuls (0,0)/(64,0), concurrent
    P = exp(S) — ONE ScalarE activation per (c6,qb,ki), [128, 1024] free
    O~/den: AV matmul lhsT=V'[h] M=128: 64 cols of v + 64 ones columns ->
       psum [128,512]: O~ on one 64-partition half, den replicated on other
    evac: DVE reciprocal_approx_fast(den half) -> rcp, DVE mul -> OT bf16
  out = proj_w @ O + proj_b (PE, interleaved with next qb; DVE evac, DMA out)

ScalarE runs ONLY exp (the structural bottleneck ~275us); everything else is
kept off it. Projections/out-proj are emitted interleaved with attention so
the PE fills its exp-wait gaps and ACT never idles after warmup.
"""
import sys
import types
from contextlib import ExitStack

import numpy as np
import ml_dtypes

import concourse.bass as bass
import concourse.mybir as mybir
import concourse.tile as tile
from concourse import bacc
from concourse.bass_utils import run_bass_kernel_spmd

# ---------------------------------------------------------------- constants
DIM = 768
H = 12
DH = 64
T = 8
TS = 8
APATCH = 196
VP = 196
B = 8
NT = APATCH * TS          # 1568 tokens per core, both q and kv side
SCALE = DH ** -0.5
NCH = DIM // 128          # 6 feature chunks (2 heads each)
KT = 13                   # k tiles: 12 full 128 + rump 32
KR = NT - 12 * 128        # 32
QB = 512                  # q block
NQB = 3                   # full q blocks; rump = 32
QR = NT - NQB * QB        # 32
F32 = mybir.dt.float32
BF16 = mybir.dt.bfloat16
ADD = mybir.AluOpType.add
MULT = mybir.AluOpType.mult

_NC_CACHE = {}

import os
V_RECIP = os.environ.get("KV_RECIP", "fast")     # fast | exact
V_MEMSET = os.environ.get("KV_MEMSET", "pool")   # pool | dve
V_RUMP = os.environ.get("KV_RUMP", "1") == "1"
V_QALL = os.environ.get("KV_QALL", "0") == "1"
V_SERIAL = os.environ.get("KV_SERIAL", "0") == "1"



def kw_of(ki):
    return 128 if ki < 12 else KR


def build_nc():
    nc = bacc.Bacc(None)

    s_xT = nc.dram_tensor("s_xT", [DIM, NT], BF16, kind="ExternalInput")
    t_xT = nc.dram_tensor("t_xT", [DIM, NT], BF16, kind="ExternalInput")
    sp_s = nc.dram_tensor("sp_s", [DIM, APATCH], BF16, kind="ExternalInput")
    tp_s = nc.dram_tensor("tp_s", [DIM, TS], BF16, kind="ExternalInput")
    sp_t = nc.dram_tensor("sp_t", [DIM, VP], BF16, kind="ExternalInput")
    tp_t = nc.dram_tensor("tp_t", [DIM, T], BF16, kind="ExternalInput")
    q_wT = nc.dram_tensor("q_wT", [DIM, DIM], BF16, kind="ExternalInput")
    k_wT = nc.dram_tensor("k_wT", [DIM, DIM], BF16, kind="ExternalInput")
    v_wT = nc.dram_tensor("v_wT", [DIM, DIM], BF16, kind="ExternalInput")
    proj_wT = nc.dram_tensor("proj_wT", [DIM, DIM], BF16, kind="ExternalInput")
    q_b2 = nc.dram_tensor("q_b2", [128, NCH], F32, kind="ExternalInput")
    k_b2 = nc.dram_tensor("k_b2", [128, NCH], F32, kind="ExternalInput")
    p_b2 = nc.dram_tensor("p_b2", [128, NCH], F32, kind="ExternalInput")
    v_br = nc.dram_tensor("v_br", [128, DIM], F32, kind="ExternalInput")
    outT = nc.dram_tensor("outT", [DIM, NT], F32, kind="ExternalOutput")
    DBG = os.environ.get("KV_DBG", "0") == "1"
    if DBG:
        dbg_qT = nc.dram_tensor("dbg_qT", [NCH, 128, NT], BF16, kind="ExternalOutput")
        dbg_kT = nc.dram_tensor("dbg_kT", [128, NT], BF16, kind="ExternalOutput")
        dbg_vP = nc.dram_tensor("dbg_vP", [128, 2, NCH, 128], BF16, kind="ExternalOutput")
        dbg_OT = nc.dram_tensor("dbg_OT", [NCH, 128, NT], BF16, kind="ExternalOutput")
        dbg_P = nc.dram_tensor("dbg_P", [128, 2, 512], BF16, kind="ExternalOutput")
        dbg_sT = nc.dram_tensor("dbg_sT", [128, NT], BF16, kind="ExternalOutput")

    with tile.TileContext(nc) as tc, ExitStack() as top:
        # ---------------- constant / persistent tiles
        cpool = top.enter_context(tc.tile_pool(name="consts", bufs=1))
        qb_t = cpool.tile([128, NCH], F32, tag="qb")
        kb_t = cpool.tile([128, NCH], F32, tag="kb")
        pb_t = cpool.tile([128, NCH], F32, tag="pb")
        vb_t = cpool.tile([128, DIM], F32, tag="vb")
        nc.sync.dma_start(qb_t[:], q_b2[:])
        nc.sync.dma_start(kb_t[:], k_b2[:])
        nc.sync.dma_start(pb_t[:], p_b2[:])
        nc.sync.dma_start(vb_t[:], v_br[:])

        w_pool = top.enter_context(tc.tile_pool(name="w", bufs=NCH))
        qw = [w_pool.tile([128, DIM], BF16, tag="qw", name=f"qw{c}") for c in range(NCH)]
        kw = [w_pool.tile([128, DIM], BF16, tag="kw", name=f"kw{c}") for c in range(NCH)]
        vw = [w_pool.tile([128, DIM], BF16, tag="vw", name=f"vw{c}") for c in range(NCH)]
        pw = [w_pool.tile([128, DIM], BF16, tag="pw", name=f"pw{c}") for c in range(NCH)]
        for c in range(NCH):
            sl = slice(c * 128, (c + 1) * 128)
            nc.gpsimd.dma_start(kw[c][:], k_wT[sl, :])
        for c in range(NCH):
            sl = slice(c * 128, (c + 1) * 128)
            nc.gpsimd.dma_start(vw[c][:], v_wT[sl, :])
            nc.sync.dma_start(qw[c][:], q_wT[sl, :])
            nc.sync.dma_start(pw[c][:], proj_wT[sl, :])

        # x + pos, bf16 feature-major
        xs_pool = top.enter_context(tc.tile_pool(name="xs", bufs=NCH))
        sT = [xs_pool.tile([128, NT], BF16, tag="sT", name=f"sT{c}") for c in range(NCH)]
        tT = [xs_pool.tile([128, NT], BF16, tag="tT", name=f"tT{c}") for c in range(NCH)]
        with ExitStack() as pr, nc.named_scope("p0_load"):
            xin_pool = pr.enter_context(tc.tile_pool(name="xin", bufs=4))
            pos_pool = pr.enter_context(tc.tile_pool(name="pos", bufs=4))
            spf_pool = pr.enter_context(tc.tile_pool(name="spf", bufs=1))
            sps_t = spf_pool.tile([128, NCH, APATCH], BF16, tag="sps")
            tps_t = spf_pool.tile([128, NCH, TS], BF16, tag="tps")
            spt_t = spf_pool.tile([128, NCH, VP], BF16, tag="spt")
            tpt_t = spf_pool.tile([128, NCH, T], BF16, tag="tpt")
            nc.scalar.dma_start(
                sps_t[:], sp_s[:].rearrange("(c p) n -> p c n", p=128))
            nc.scalar.dma_start(
                tps_t[:], tp_s[:].rearrange("(c p) n -> p c n", p=128))
            nc.scalar.dma_start(
                spt_t[:], sp_t[:].rearrange("(c p) n -> p c n", p=128))
            nc.scalar.dma_start(
                tpt_t[:], tp_t[:].rearrange("(c p) n -> p c n", p=128))

            def build_pos(pt, c, space_t, temp_t, nsp, ntp):
                a = space_t[:, c, :, None]
                b = temp_t[:, c, None, :]
                a2, b2 = bass.broadcast_tensor_aps(a, b)
                nc.vector.tensor_tensor(
                    pt[:].rearrange("p (n t) -> p n t", t=ntp), a2, b2, ADD)

            for c in range(NCH):
                sl = slice(c * 128, (c + 1) * 128)
                xt = xin_pool.tile([128, NT], BF16, tag="xin", name=f"xt{c}")
                pt = pos_pool.tile([128, NT], BF16, tag="pos", name=f"pt{c}")
                nc.gpsimd.dma_start(xt[:], t_xT[sl, :])
                build_pos(pt, c, spt_t, tpt_t, VP, T)
                nc.vector.tensor_add(tT[c][:], xt[:], pt[:])
            for c in range(NCH):
                sl = slice(c * 128, (c + 1) * 128)
                xs2 = xin_pool.tile([128, NT], BF16, tag="xin", name=f"xs{c}")
                ps2 = pos_pool.tile([128, NT], BF16, tag="pos", name=f"ps{c}")
                nc.sync.dma_start(xs2[:], s_xT[sl, :])
                build_pos(ps2, c, sps_t, tps_t, APATCH, TS)
                nc.vector.tensor_add(sT[c][:], xs2[:], ps2[:])

        # q/k feature-major bf16; V' token-major bf16 with ones blocks
        qkT_pool = top.enter_context(tc.tile_pool(name="qkT", bufs=NCH))
        qT = [qkT_pool.tile([128, NT], BF16, tag="qT", name=f"qT{c}") for c in range(NCH)]
        kT = [qkT_pool.tile([128, NT], BF16, tag="kT", name=f"kT{c}") for c in range(NCH)]
        vP_pool = top.enter_context(tc.tile_pool(name="vP", bufs=KT))
        # layout: [tok, parity, pair, 128]; head h = 2*pair+parity
        vP = [vP_pool.tile([128, 2, NCH, 128], BF16, tag="vP", name=f"vP{k}")
              for k in range(KT)]
        ms_eng = nc.gpsimd if V_MEMSET == "pool" else nc.vector
        for k in range(KT):
            kwid = kw_of(k)
            ms_eng.memset(vP[k][:kwid, :, :, 0:64], 1.0)

        OT_pool = top.enter_context(tc.tile_pool(name="OT", bufs=NCH))
        OT = [OT_pool.tile([128, NT], BF16, tag="OT", name=f"OT{c}") for c in range(NCH)]

        # ---------------- psum pools (8 banks total)
        qk_psum = top.enter_context(tc.tile_pool(name="qkps", bufs=2, space="PSUM"))
        av_psum = top.enter_context(tc.tile_pool(name="avps", bufs=2, space="PSUM"))
        gm_psum = top.enter_context(tc.tile_pool(name="gmps", bufs=2, space="PSUM"))

        P_pool = top.enter_context(tc.tile_pool(name="P", bufs=3))
        rcp_pool = top.enter_context(tc.tile_pool(name="rcp", bufs=2))
        ost_pool = top.enter_context(tc.tile_pool(name="ost", bufs=2))

        QSL = [(i * QB, QB) for i in range(NQB)] + [(NQB * QB, QR)]

        def emit_proj(ws, xsrc, dst, bias_t, c_out, q0, qn):
            """dst[c_out][:, q0:q0+qn] = ws.T @ x (+bias), bf16 evac on DVE."""
            ps = gm_psum.tile([128, 512], F32, tag="gm")
            for c in range(NCH):
                nc.tensor.matmul(
                    ps[:, 0:qn],
                    ws[c][:, c_out * 128:(c_out + 1) * 128],
                    xsrc[c][:, q0:q0 + qn],
                    start=(c == 0), stop=(c == NCH - 1),
                )
            nc.vector.tensor_scalar_add(
                dst[c_out][:, q0:q0 + qn], ps[:, 0:qn], bias_t[:, c_out:c_out + 1]
            )

        def emit_vproj(k):
            """V' for k-tile k. v_wT cols pre-reordered on host:
            group0 = even heads' v dims, group1 = odd heads'."""
            kwid = kw_of(k)
            for g in range(2):
                ps = gm_psum.tile([128, 512], F32, tag="gm")
                for c in range(NCH):
                    nc.tensor.matmul(
                        ps[:kwid, 0:384],
                        tT[c][:, k * 128:k * 128 + kwid],
                        vw[c][:, g * 384:(g + 1) * 384],
                        start=(c == 0), stop=(c == NCH - 1),
                    )
                dst = vP[k][:kwid, g, :, 64:128]
                src = ps[:kwid, 0:384].rearrange("p (h d) -> p h d", d=DH)
                bia = vb_t[:kwid, g * 384:(g + 1) * 384].rearrange(
                    "p (h d) -> p h d", d=DH)
                nc.vector.tensor_tensor(dst, src, bia, ADD)

        def emit_outproj(c_out, q0, qn):
            ps = gm_psum.tile([128, 512], F32, tag="gm")
            for c in range(NCH):
                nc.tensor.matmul(
                    ps[:, 0:qn],
                    pw[c][:, c_out * 128:(c_out + 1) * 128],
                    OT[c][:, q0:q0 + qn],
                    start=(c == 0), stop=(c == NCH - 1),
                )
            oe = ost_pool.tile([128, 512], F32, tag="ost")
            nc.vector.tensor_scalar_add(
                oe[:, 0:qn], ps[:, 0:qn], pb_t[:, c_out:c_out + 1]
            )
            nc.sync.dma_start(outT[c_out * 128:(c_out + 1) * 128, q0:q0 + qn],
                              oe[:, 0:qn])

        filler = []  # deferred out-proj emissions (no forward PE deps)

        def filler_emit(n):
            for _ in range(n):
                if filler:
                    filler.pop(0)()

        def av_evac(c6, avA, avB, q0, qn):
            """Normalize + evac both heads of chunk c6 for q slice [q0, q0+qn)."""
            # both heads: den replicated at psum parts 0:64 (base-0 for the
            # custom DVE recip), O~ at 64:128; rcp written at base-0 SBUF.
            rcp = rcp_pool.tile([128, 1024], F32, tag="rcp")
            recip = (nc.vector.reciprocal_approx_fast if V_RECIP == "fast"
                     else nc.vector.reciprocal)
            recip(rcp[0:64, 0:qn], avA[0:64, 0:qn])
            recip(rcp[0:64, 512:512 + qn], avB[0:64, 0:qn])
            nc.vector.tensor_tensor(
                OT[c6][0:64, q0:q0 + qn], avA[64:128, 0:qn], rcp[0:64, 0:qn],
                MULT)
            nc.vector.tensor_tensor(
                OT[c6][64:128, q0:q0 + qn], avB[64:128, 0:qn],
                rcp[0:64, 512:512 + qn], MULT)

        def qk_mm(c6, ki, q0, qn):
            kwid = kw_of(ki)
            ksl = slice(ki * 128, ki * 128 + kwid)
            qk = qk_psum.tile([128, 1024], F32, tag="qk", name=f"qk{ki % 2}")
            nc.tensor.matmul(
                qk[:kwid, 0:qn], kT[c6][0:64, ksl], qT[c6][0:64, q0:q0 + qn],
                start=True, stop=True, tile_position=(0, 0),
            )
            nc.tensor.matmul(
                qk[:kwid, 512:512 + qn], kT[c6][64:128, ksl],
                qT[c6][64:128, q0:q0 + qn],
                start=True, stop=True, tile_position=(64, 0),
            )
            return qk

        # ---------------- main attention pipeline
        pend = []
        with nc.named_scope("attn"):
            if V_SERIAL:
                for k in range(KT):
                    emit_vproj(k)
            for qbi in range(NQB):
                q0 = qbi * QB
                for c6 in range(NCH):
                    # required projections for THIS (qbi, c6), in PE order
                    if V_SERIAL:
                        filler_emit(1)
                    if qbi == 0:
                        if V_SERIAL or V_QALL:
                            for (a, b) in QSL:
                                emit_proj(kw, tT, kT, kb_t, c6, a, b)
                            for (a, b) in (QSL if V_QALL else [(0, QB)]):
                                emit_proj(qw, sT, qT, qb_t, c6, a, b)
                        elif c6 == 0:
                            # critical path: only what QK(ki=0..3) needs
                            emit_proj(kw, tT, kT, kb_t, 0, 0, QB)
                            emit_proj(qw, sT, qT, qb_t, 0, 0, QB)
                            emit_vproj(0)
                            emit_vproj(1)
                        # seed next chunk's projections, paced inside ki loop
                        if not (V_SERIAL or V_QALL) and c6 + 1 < NCH:
                            for (a, b) in QSL:
                                pend.append(
                                    (lambda c=c6 + 1, a=a, b=b:
                                     emit_proj(kw, tT, kT, kb_t, c, a, b)))
                            pend.append(
                                (lambda c=c6 + 1:
                                 emit_proj(qw, sT, qT, qb_t, c, 0, QB)))
                    else:
                        if not V_QALL:
                            if qbi == 1:
                                emit_proj(qw, sT, qT, qb_t, c6, QB, QB)
                            if qbi == 2:
                                emit_proj(qw, sT, qT, qb_t, c6, 2 * QB, QB)
                        filler.append(
                            (lambda c=c6, a=(qbi - 1) * QB:
                             emit_outproj(c, a, QB)))

                    avA = av_psum.tile([128, 512], F32, tag="av", name="avA")
                    avB = av_psum.tile([128, 512], F32, tag="av", name="avB")

                    qk_next = qk_mm(c6, 0, q0, QB)
                    for ki in range(KT):
                        kwid = kw_of(ki)
                        qk = qk_next
                        if V_SERIAL:
                            pass
                        elif qbi == 0:
                            if c6 == 0:
                                if ki + 2 < KT:
                                    emit_vproj(ki + 2)
                                if ki in (1, 4, 7):
                                    a = (ki + 2) // 3 * QB
                                    emit_proj(kw, tT, kT, kb_t, 0, a,
                                              QB if a < NQB * QB else QR)
                                if pend and ki in (3, 5, 9, 11, 12):
                                    pend.pop(0)()
                            elif pend and ki % 2 == 1:
                                pend.pop(0)()
                        elif filler and (ki % 4 == 3):
                            filler_emit(1)
                        if ki + 1 < KT:
                            qk_next = qk_mm(c6, ki + 1, q0, QB)
                        ex = P_pool.tile([128, 2, 512], BF16, tag="P")
                        nc.scalar.activation(
                            ex[:kwid, :, :],
                            qk[:kwid, :].rearrange("p (h q) -> p h q", h=2),
                            mybir.ActivationFunctionType.Exp,
                        )
                        if DBG and qbi == 0 and c6 == 0 and ki == 0:
                            nc.sync.dma_start(dbg_P[:], ex[:, :, :])
                        for hh in range(2):
                            nc.tensor.matmul(
                                (avA if hh == 0 else avB)[:, 0:QB],
                                vP[ki][:kwid, hh, c6, :],
                                ex[:kwid, hh, :],
                                start=(ki == 0), stop=(ki == KT - 1),
                            )
                    av_evac(c6, avA, avB, q0, QB)

            # ---- rump q block (32 cols): batch QK psum across all ki
            q0 = NQB * QB
            with nc.named_scope("rump"):
                for c6 in (range(NCH) if V_RUMP else []):
                    if not V_QALL:
                        emit_proj(qw, sT, qT, qb_t, c6, q0, QR)
                    avA = av_psum.tile([128, 512], F32, tag="av", name="avA")
                    avB = av_psum.tile([128, 512], F32, tag="av", name="avB")
                    qk = qk_psum.tile([128, 1024], F32, tag="qk", name="qkr")
                    # head-major, ki padded to 16: head A fills psum bank 0,
                    # head B bank 1 (concurrent pair must hit distinct banks)
                    qkv = qk[:, :].rearrange("p (h k q) -> p h k q", h=2, k=16)
                    for ki in range(KT):
                        kwid = kw_of(ki)
                        ksl = slice(ki * 128, ki * 128 + kwid)
                        nc.tensor.matmul(
                            qkv[:kwid, 0, ki, :], kT[c6][0:64, ksl],
                            qT[c6][0:64, q0:q0 + QR],
                            start=True, stop=True, tile_position=(0, 0),
                        )
                        nc.tensor.matmul(
                            qkv[:kwid, 1, ki, :], kT[c6][64:128, ksl],
                            qT[c6][64:128, q0:q0 + QR],
                            start=True, stop=True, tile_position=(64, 0),
                        )
                    ex = P_pool.tile([128, 2, KT, QR], BF16, tag="Pr")
                    for hh in range(2):
                        nc.scalar.activation(
                            ex[:, hh, 0:12, :], qkv[:, hh, 0:12, :],
                            mybir.ActivationFunctionType.Exp,
                        )
                        nc.scalar.activation(
                            ex[0:KR, hh, 12, :], qkv[0:KR, hh, 12, :],
                            mybir.ActivationFunctionType.Exp,
                        )
                    filler_emit(2)
                    for ki in range(KT):
                        kwid = kw_of(ki)
                        for hh in range(2):
                            nc.tensor.matmul(
                                (avA if hh == 0 else avB)[:, 0:QR],
                                vP[ki][:kwid, hh, c6, :],
                                ex[:kwid, hh, ki, :],
                                start=(ki == 0), stop=(ki == KT - 1),
                            )
                    av_evac(c6, avA, avB, q0, QR)
                    filler.append(
                        (lambda c=c6, a=(NQB - 1) * QB:
                         emit_outproj(c, a, QB)))

            if DBG:
                nc.sync.dma_start(dbg_sT[:], sT[0][:])
                for c in range(NCH):
                    nc.sync.dma_start(dbg_qT[c], qT[c][:])
                nc.sync.dma_start(dbg_kT[:], kT[0][:])
                nc.sync.dma_start(dbg_vP[:], vP[0][:])
                for c in range(NCH):
                    nc.sync.dma_start(dbg_OT[c], OT[c][:])
            # ---- drain remaining filler + final out-proj slices
            with nc.named_scope("tail"):
                filler_emit(len(filler))
                if V_RUMP:
                    for c6 in range(NCH):
                        emit_outproj(c6, NQB * QB, QR)

    nc.finalize()
    return nc


def _install_axon_ntff_shim():
    if "antenv.axon_hooks" in sys.modules:
        return
    mod = types.ModuleType("antenv.axon_hooks")
    mod._hook = None
    mod.set_axon_ntff_profile_hook = lambda h: setattr(mod, "_hook", h)
    mod.get_axon_ntff_profile_hook = lambda: mod._hook
    sys.modules["antenv.axon_hooks"] = mod
    try:
        import antenv

        antenv.axon_hooks = mod
        from trn_agent_boot.trn_boot import _ntff_profile_via_ctypes

        hook = _ntff_profile_via_ctypes("/opt/axon/libaxon_pjrt.so")
        if hook is not None:
            mod.set_axon_ntff_profile_hook(hook)
    except Exception:
        pass


def prep_inputs(s_x, t_x, clip_space_pos, vmae_space_pos, clip_temporal_pos,
                vmae_temporal_pos, q_w, q_b, kv_w, kv_b, proj_w, proj_b):
    """Host-side sharding/layout prep. Returns list of 8 per-core input maps."""
    f = np.float32
    bf = ml_dtypes.bfloat16
    sp_s = np.ascontiguousarray(np.asarray(clip_space_pos).T).astype(bf)
    tp_s = np.ascontiguousarray(np.asarray(clip_temporal_pos).T).astype(bf)
    sp_t = np.ascontiguousarray(np.asarray(vmae_space_pos).T).astype(bf)
    tp_t = np.ascontiguousarray(np.asarray(vmae_temporal_pos).T).astype(bf)
    q_wT = np.ascontiguousarray(np.asarray(q_w).T * SCALE).astype(bf)
    k_wT = np.ascontiguousarray(np.asarray(kv_w)[:DIM].T).astype(bf)
    # v weight rows reordered: [even heads' v dims | odd heads' v dims]
    v_w = np.asarray(kv_w)[DIM:]         # [768 out, 768 in]
    v_b = np.asarray(kv_b)[DIM:]
    order = np.concatenate([
        np.arange(DIM).reshape(H, DH)[0::2].reshape(-1),
        np.arange(DIM).reshape(H, DH)[1::2].reshape(-1),
    ])
    v_wT = np.ascontiguousarray(v_w[order].T).astype(bf)
    v_br = np.ascontiguousarray(
        np.broadcast_to(v_b[order].reshape(1, DIM), (128, DIM)), dtype=f)
    proj_wT = np.ascontiguousarray(np.asarray(proj_w).T).astype(bf)
    q_b2 = np.ascontiguousarray(
        (np.asarray(q_b) * SCALE).reshape(NCH, 128).T, dtype=f)
    k_b2 = np.ascontiguousarray(
        np.asarray(kv_b)[:DIM].reshape(NCH, 128).T, dtype=f)
    p_b2 = np.ascontiguousarray(np.asarray(proj_b).reshape(NCH, 128).T, dtype=f)

    in_maps = []
    for b in range(B):
        s_slice = np.asarray(s_x)[:, b * TS:(b + 1) * TS, :]  # (196, 8, 768)
        t_slice = np.asarray(t_x)[1:, b * T:(b + 1) * T, :]   # (196, 8, 768)
        s_xT = np.ascontiguousarray(
            s_slice.transpose(2, 0, 1).reshape(DIM, NT)).astype(bf)
        t_xT = np.ascontiguousarray(
            t_slice.transpose(2, 0, 1).reshape(DIM, NT)).astype(bf)
        in_maps.append({
            "s_xT": s_xT, "t_xT": t_xT,
            "sp_s": sp_s, "tp_s": tp_s, "sp_t": sp_t, "tp_t": tp_t,
            "q_wT": q_wT, "k_wT": k_wT, "v_wT": v_wT, "proj_wT": proj_wT,
            "q_b2": q_b2, "k_b2": k_b2, "p_b2": p_b2, "v_br": v_br,
        })
    return in_maps


def unshard_output(results):
    """results: list of 8 dicts with 'outT' [768, 1568] -> (196, 64, 768)."""
    out = np.empty((APATCH, B * TS, DIM), dtype=np.float32)
    for b in range(B):
        o = results[b]["outT"].reshape(DIM, APATCH, TS)
        out[:, b * TS:(b + 1) * TS, :] = o.transpose(1, 2, 0)
    return out


def kernel(**inputs):
    _install_axon_ntff_shim()
    in_maps = prep_inputs(**inputs)
    if "nc" not in _NC_CACHE:
        _NC_CACHE["nc"] = build_nc()
    nc = _NC_CACHE["nc"]
    res = run_bass_kernel_spmd(nc, in_maps, core_ids=list(range(B)))
    return unshard_output(res.results)


if __name__ == "__main__":
    rng = np.random.default_rng(0)
    fake = {
        "s_x": rng.standard_normal((APATCH, B * TS, DIM), dtype=np.float32),
        "t_x": rng.standard_normal((VP + 1, B * T, DIM), dtype=np.float32),
        "clip_space_pos": SCALE * rng.standard_normal((APATCH, DIM), dtype=np.float32),
        "vmae_space_pos": SCALE * rng.standard_normal((VP, DIM), dtype=np.float32),
        "clip_temporal_pos": SCALE * rng.standard_normal((TS, DIM), dtype=np.float32),
        "vmae_temporal_pos": SCALE * rng.standard_normal((T, DIM), dtype=np.float32),
        "q_w": (0.02 * rng.standard_normal((DIM, DIM))).astype(np.float32),
        "q_b": np.zeros(DIM, np.float32),
        "kv_w": (0.02 * rng.standard_normal((2 * DIM, DIM))).astype(np.float32),
        "kv_b": np.zeros(2 * DIM, np.float32),
        "proj_w": (0.02 * rng.standard_normal((DIM, DIM))).astype(np.float32),
        "proj_b": np.zeros(DIM, np.float32),
    }
    out = kernel(**fake)
    print("out", out.shape, out.dtype)


# revision 18
# speedup vs baseline: 1.4678x; 1.0094x over previous
"""Trainium2 Bass kernel for nn_CrossAttentionT2S (fused pos-embed cross-attention).

Sharding: data-parallel over the true batch axis b=8, one batch element per
NeuronCore. All tensors bf16 on device; feature-major ("transposed",
[feature, token]) layouts so matmuls contract over the partition dim.

Per core (NT=1568 q tokens, 1568 kv tokens, 12 heads, dh=64):
  tT = t_xT + pos_tT ; sT = s_xT + pos_sT          (DVE bf16 adds)
  kT = k_w @ t (feature-major, 6 chunks of 2 heads) (PE, evac on DVE + bias)
  qT = (q_w*SCALE) @ s + q_b*SCALE                  (PE, evac DVE)
  V' = token-major [128tok, parity, 6, 128]: even heads [v|ones64],
       odd heads [ones64|v]                          (PE, evac DVE, ones memset)
  per (qb in 512,512,512 + rump32, c6 head-pair, ki in 13):
    S[k128, q512]x2 heads — two row-tiled matmuls (0,0)/(64,0), concurrent
    P = exp(S) — ONE ScalarE activation per (c6,qb,ki), [128, 1024] free
    O~/den: AV matmul lhsT=V'[h] M=128: 64 cols of v + 64 ones columns ->
       psum [128,512]: O~ on one 64-partition half, den replicated on other
    evac: DVE reciprocal_approx_fast(den half) -> rcp, DVE mul -> OT bf16
  out = proj_w @ O + proj_b (PE, interleaved with next qb; DVE evac, DMA out)

ScalarE runs ONLY exp (the structural bottleneck ~275us); everything else is
kept off it. Projections/out-proj are emitted interleaved with attention so
the PE fills its exp-wait gaps and ACT never idles after warmup.
"""
import sys
import types
from contextlib import ExitStack

import numpy as np
import ml_dtypes

import concourse.bass as bass
import concourse.mybir as mybir
import concourse.tile as tile
from concourse import bacc
from concourse.bass_utils import run_bass_kernel_spmd

# ---------------------------------------------------------------- constants
DIM = 768
H = 12
DH = 64
T = 8
TS = 8
APATCH = 196
VP = 196
B = 8
NT = APATCH * TS          # 1568 tokens per core, both q and kv side
SCALE = DH ** -0.5
NCH = DIM // 128          # 6 feature chunks (2 heads each)
KT = 13                   # k tiles: 12 full 128 + rump 32
KR = NT - 12 * 128        # 32
QB = 512                  # q block
NQB = 3                   # full q blocks; rump = 32
QR = NT - NQB * QB        # 32
F32 = mybir.dt.float32
BF16 = mybir.dt.bfloat16
ADD = mybir.AluOpType.add
MULT = mybir.AluOpType.mult

_NC_CACHE = {}

import os
V_RECIP = os.environ.get("KV_RECIP", "fast")     # fast | exact
V_MEMSET = os.environ.get("KV_MEMSET", "pool")   # pool | dve
V_RUMP = os.environ.get("KV_RUMP", "1") == "1"
V_QALL = os.environ.get("KV_QALL", "0") == "1"
V_SERIAL = os.environ.get("KV_SERIAL", "0") == "1"



def kw_of(ki):
    return 128 if ki < 12 else KR


def build_nc():
    nc = bacc.Bacc(None)

    s_xT = nc.dram_tensor("s_xT", [DIM, NT], BF16, kind="ExternalInput")
    t_xT = nc.dram_tensor("t_xT", [DIM, NT], BF16, kind="ExternalInput")
    sp_s = nc.dram_tensor("sp_s", [DIM, APATCH], BF16, kind="ExternalInput")
    tp_s = nc.dram_tensor("tp_s", [DIM, TS], BF16, kind="ExternalInput")
    sp_t = nc.dram_tensor("sp_t", [DIM, VP], BF16, kind="ExternalInput")
    tp_t = nc.dram_tensor("tp_t", [DIM, T], BF16, kind="ExternalInput")
    q_wT = nc.dram_tensor("q_wT", [DIM, DIM], BF16, kind="ExternalInput")
    k_wT = nc.dram_tensor("k_wT", [DIM, DIM], BF16, kind="ExternalInput")
    v_wT = nc.dram_tensor("v_wT", [DIM, DIM], BF16, kind="ExternalInput")
    proj_wT = nc.dram_tensor("proj_wT", [DIM, DIM], BF16, kind="ExternalInput")
    q_b2 = nc.dram_tensor("q_b2", [128, NCH], F32, kind="ExternalInput")
    k_b2 = nc.dram_tensor("k_b2", [128, NCH], F32, kind="ExternalInput")
    p_b2 = nc.dram_tensor("p_b2", [128, NCH], F32, kind="ExternalInput")
    v_br = nc.dram_tensor("v_br", [128, DIM], F32, kind="ExternalInput")
    outT = nc.dram_tensor("outT", [DIM, NT], F32, kind="ExternalOutput")
    DBG = os.environ.get("KV_DBG", "0") == "1"
    if DBG:
        dbg_qT = nc.dram_tensor("dbg_qT", [NCH, 128, NT], BF16, kind="ExternalOutput")
        dbg_kT = nc.dram_tensor("dbg_kT", [128, NT], BF16, kind="ExternalOutput")
        dbg_vP = nc.dram_tensor("dbg_vP", [128, 2, NCH, 128], BF16, kind="ExternalOutput")
        dbg_OT = nc.dram_tensor("dbg_OT", [NCH, 128, NT], BF16, kind="ExternalOutput")
        dbg_P = nc.dram_tensor("dbg_P", [128, 2, 512], BF16, kind="ExternalOutput")
        dbg_sT = nc.dram_tensor("dbg_sT", [128, NT], BF16, kind="ExternalOutput")

    with tile.TileContext(nc) as tc, ExitStack() as top:
        # ---------------- constant / persistent tiles
        cpool = top.enter_context(tc.tile_pool(name="consts", bufs=1))
        qb_t = cpool.tile([128, NCH], F32, tag="qb")
        kb_t = cpool.tile([128, NCH], F32, tag="kb")
        pb_t = cpool.tile([128, NCH], F32, tag="pb")
        vb_t = cpool.tile([128, DIM], F32, tag="vb")
        nc.sync.dma_start(qb_t[:], q_b2[:])
        nc.sync.dma_start(kb_t[:], k_b2[:])
        nc.sync.dma_start(pb_t[:], p_b2[:])
        nc.sync.dma_start(vb_t[:], v_br[:])

        w_pool = top.enter_context(tc.tile_pool(name="w", bufs=NCH))
        qw = [w_pool.tile([128, DIM], BF16, tag="qw", name=f"qw{c}") for c in range(NCH)]
        kw = [w_pool.tile([128, DIM], BF16, tag="kw", name=f"kw{c}") for c in range(NCH)]
        vw = [w_pool.tile([128, DIM], BF16, tag="vw", name=f"vw{c}") for c in range(NCH)]
        pw = [w_pool.tile([128, DIM], BF16, tag="pw", name=f"pw{c}") for c in range(NCH)]
        for c in range(NCH):
            sl = slice(c * 128, (c + 1) * 128)
            nc.sync.dma_start(kw[c][:], k_wT[sl, :])

        # x + pos, bf16 feature-major
        xs_pool = top.enter_context(tc.tile_pool(name="xs", bufs=NCH))
        sT = [xs_pool.tile([128, NT], BF16, tag="sT", name=f"sT{c}") for c in range(NCH)]
        tT = [xs_pool.tile([128, NT], BF16, tag="tT", name=f"tT{c}") for c in range(NCH)]
        with ExitStack() as pr, nc.named_scope("p0_load"):
            xin_pool = pr.enter_context(tc.tile_pool(name="xin", bufs=4))
            pos_pool = pr.enter_context(tc.tile_pool(name="pos", bufs=4))
            spf_pool = pr.enter_context(tc.tile_pool(name="spf", bufs=1))
            sps_t = spf_pool.tile([128, NCH, APATCH], BF16, tag="sps")
            tps_t = spf_pool.tile([128, NCH, TS], BF16, tag="tps")
            spt_t = spf_pool.tile([128, NCH, VP], BF16, tag="spt")
            tpt_t = spf_pool.tile([128, NCH, T], BF16, tag="tpt")
            nc.scalar.dma_start(
                sps_t[:], sp_s[:].rearrange("(c p) n -> p c n", p=128))
            nc.scalar.dma_start(
                tps_t[:], tp_s[:].rearrange("(c p) n -> p c n", p=128))
            nc.scalar.dma_start(
                spt_t[:], sp_t[:].rearrange("(c p) n -> p c n", p=128))
            nc.scalar.dma_start(
                tpt_t[:], tp_t[:].rearrange("(c p) n -> p c n", p=128))

            def build_pos(pt, c, space_t, temp_t, nsp, ntp):
                a = space_t[:, c, :, None]
                b = temp_t[:, c, None, :]
                a2, b2 = bass.broadcast_tensor_aps(a, b)
                nc.vector.tensor_tensor(
                    pt[:].rearrange("p (n t) -> p n t", t=ntp), a2, b2, ADD)

            for c in range(NCH):
                sl = slice(c * 128, (c + 1) * 128)
                xt = xin_pool.tile([128, NT], BF16, tag="xin", name=f"xt{c}")
                pt = pos_pool.tile([128, NT], BF16, tag="pos", name=f"pt{c}")
                nc.gpsimd.dma_start(xt[:], t_xT[sl, :])
                build_pos(pt, c, spt_t, tpt_t, VP, T)
                nc.vector.tensor_add(tT[c][:], xt[:], pt[:])
            for c in range(NCH):
                sl = slice(c * 128, (c + 1) * 128)
                xs2 = xin_pool.tile([128, NT], BF16, tag="xin", name=f"xs{c}")
                ps2 = pos_pool.tile([128, NT], BF16, tag="pos", name=f"ps{c}")
                nc.sync.dma_start(xs2[:], s_xT[sl, :])
                build_pos(ps2, c, sps_t, tps_t, APATCH, TS)
                nc.vector.tensor_add(sT[c][:], xs2[:], ps2[:])
            # lower-priority weights after the x/pos critical path
            for c in range(NCH):
                sl = slice(c * 128, (c + 1) * 128)
                nc.gpsimd.dma_start(vw[c][:], v_wT[sl, :])
                nc.sync.dma_start(qw[c][:], q_wT[sl, :])
                nc.scalar.dma_start(pw[c][:], proj_wT[sl, :])

        # q/k feature-major bf16; V' token-major bf16 with ones blocks
        qkT_pool = top.enter_context(tc.tile_pool(name="qkT", bufs=NCH))
        qT = [qkT_pool.tile([128, NT], BF16, tag="qT", name=f"qT{c}") for c in range(NCH)]
        kT = [qkT_pool.tile([128, NT], BF16, tag="kT", name=f"kT{c}") for c in range(NCH)]
        vP_pool = top.enter_context(tc.tile_pool(name="vP", bufs=KT))
        # layout: [tok, parity, pair, 128]; head h = 2*pair+parity
        vP = [vP_pool.tile([128, 2, NCH, 128], BF16, tag="vP", name=f"vP{k}")
              for k in range(KT)]
        ms_eng = nc.gpsimd if V_MEMSET == "pool" else nc.vector
        for k in range(KT):
            kwid = kw_of(k)
            ms_eng.memset(vP[k][:kwid, :, :, 0:64], 1.0)

        OT_pool = top.enter_context(tc.tile_pool(name="OT", bufs=NCH))
        OT = [OT_pool.tile([128, NT], BF16, tag="OT", name=f"OT{c}") for c in range(NCH)]

        # ---------------- psum pools (8 banks total)
        qk_psum = top.enter_context(tc.tile_pool(name="qkps", bufs=2, space="PSUM"))
        av_psum = top.enter_context(tc.tile_pool(name="avps", bufs=2, space="PSUM"))
        gm_psum = top.enter_context(tc.tile_pool(name="gmps", bufs=2, space="PSUM"))

        P_pool = top.enter_context(tc.tile_pool(name="P", bufs=3))
        rcp_pool = top.enter_context(tc.tile_pool(name="rcp", bufs=2))
        ost_pool = top.enter_context(tc.tile_pool(name="ost", bufs=2))

        QSL = [(i * QB, QB) for i in range(NQB)] + [(NQB * QB, QR)]

        def emit_proj(ws, xsrc, dst, bias_t, c_out, q0, qn):
            """dst[c_out][:, q0:q0+qn] = ws.T @ x (+bias), bf16 evac on DVE."""
            ps = gm_psum.tile([128, 512], F32, tag="gm")
            for c in range(NCH):
                nc.tensor.matmul(
                    ps[:, 0:qn],
                    ws[c][:, c_out * 128:(c_out + 1) * 128],
                    xsrc[c][:, q0:q0 + qn],
                    start=(c == 0), stop=(c == NCH - 1),
                )
            nc.vector.tensor_scalar_add(
                dst[c_out][:, q0:q0 + qn], ps[:, 0:qn], bias_t[:, c_out:c_out + 1]
            )

        def emit_vproj(k):
            """V' for k-tile k. v_wT cols pre-reordered on host:
            group0 = even heads' v dims, group1 = odd heads'."""
            kwid = kw_of(k)
            for g in range(2):
                ps = gm_psum.tile([128, 512], F32, tag="gm")
                for c in range(NCH):
                    nc.tensor.matmul(
                        ps[:kwid, 0:384],
                        tT[c][:, k * 128:k * 128 + kwid],
                        vw[c][:, g * 384:(g + 1) * 384],
                        start=(c == 0), stop=(c == NCH - 1),
                    )
                dst = vP[k][:kwid, g, :, 64:128]
                src = ps[:kwid, 0:384].rearrange("p (h d) -> p h d", d=DH)
                bia = vb_t[:kwid, g * 384:(g + 1) * 384].rearrange(
                    "p (h d) -> p h d", d=DH)
                nc.vector.tensor_tensor(dst, src, bia, ADD)

        def emit_outproj(c_out, q0, qn):
            ps = gm_psum.tile([128, 512], F32, tag="gm")
            for c in range(NCH):
                nc.tensor.matmul(
                    ps[:, 0:qn],
                    pw[c][:, c_out * 128:(c_out + 1) * 128],
                    OT[c][:, q0:q0 + qn],
                    start=(c == 0), stop=(c == NCH - 1),
                )
            oe = ost_pool.tile([128, 512], F32, tag="ost")
            nc.vector.tensor_scalar_add(
                oe[:, 0:qn], ps[:, 0:qn], pb_t[:, c_out:c_out + 1]
            )
            nc.sync.dma_start(outT[c_out * 128:(c_out + 1) * 128, q0:q0 + qn],
                              oe[:, 0:qn])

        filler = []  # deferred out-proj emissions (no forward PE deps)

        def filler_emit(n):
            for _ in range(n):
                if filler:
                    filler.pop(0)()

        def av_evac(c6, avA, avB, q0, qn):
            """Normalize + evac both heads of chunk c6 for q slice [q0, q0+qn)."""
            # both heads: den replicated at psum parts 0:64 (base-0 for the
            # custom DVE recip), O~ at 64:128; rcp written at base-0 SBUF.
            rcp = rcp_pool.tile([128, 1024], F32, tag="rcp")
            recip = (nc.vector.reciprocal_approx_fast if V_RECIP == "fast"
                     else nc.vector.reciprocal)
            recip(rcp[0:64, 0:qn], avA[0:64, 0:qn])
            recip(rcp[0:64, 512:512 + qn], avB[0:64, 0:qn])
            nc.vector.tensor_tensor(
                OT[c6][0:64, q0:q0 + qn], avA[64:128, 0:qn], rcp[0:64, 0:qn],
                MULT)
            nc.vector.tensor_tensor(
                OT[c6][64:128, q0:q0 + qn], avB[64:128, 0:qn],
                rcp[0:64, 512:512 + qn], MULT)

        def qk_mm(c6, ki, q0, qn):
            kwid = kw_of(ki)
            ksl = slice(ki * 128, ki * 128 + kwid)
            qk = qk_psum.tile([128, 1024], F32, tag="qk", name=f"qk{ki % 2}")
            nc.tensor.matmul(
                qk[:kwid, 0:qn], kT[c6][0:64, ksl], qT[c6][0:64, q0:q0 + qn],
                start=True, stop=True, tile_position=(0, 0),
            )
            nc.tensor.matmul(
                qk[:kwid, 512:512 + qn], kT[c6][64:128, ksl],
                qT[c6][64:128, q0:q0 + qn],
                start=True, stop=True, tile_position=(64, 0),
            )
            return qk

        # ---------------- main attention pipeline
        pend = []
        with nc.named_scope("attn"):
            if V_SERIAL:
                for k in range(KT):
                    emit_vproj(k)
            for qbi in range(NQB):
                q0 = qbi * QB
                for c6 in range(NCH):
                    # required projections for THIS (qbi, c6), in PE order
                    if V_SERIAL:
                        filler_emit(1)
                    if qbi == 0:
                        if V_SERIAL or V_QALL:
                            for (a, b) in QSL:
                                emit_proj(kw, tT, kT, kb_t, c6, a, b)
                            for (a, b) in (QSL if V_QALL else [(0, QB)]):
                                emit_proj(qw, sT, qT, qb_t, c6, a, b)
                        elif c6 == 0:
                            # critical path: only what QK(ki=0..3) needs
                            emit_proj(kw, tT, kT, kb_t, 0, 0, QB)
                            emit_proj(qw, sT, qT, qb_t, 0, 0, QB)
                            emit_vproj(0)
                            emit_vproj(1)
                        # seed next chunk's projections, paced inside ki loop
                        if not (V_SERIAL or V_QALL) and c6 + 1 < NCH:
                            for (a, b) in QSL:
                                pend.append(
                                    (lambda c=c6 + 1, a=a, b=b:
                                     emit_proj(kw, tT, kT, kb_t, c, a, b)))
                            pend.append(
                                (lambda c=c6 + 1:
                                 emit_proj(qw, sT, qT, qb_t, c, 0, QB)))
                    else:
                        if not V_QALL:
                            if qbi == 1:
                                emit_proj(qw, sT, qT, qb_t, c6, QB, QB)
                            if qbi == 2:
                                emit_proj(qw, sT, qT, qb_t, c6, 2 * QB, QB)
                        filler.append(
                            (lambda c=c6, a=(qbi - 1) * QB:
                             emit_outproj(c, a, QB)))

                    avA = av_psum.tile([128, 512], F32, tag="av", name="avA")
                    avB = av_psum.tile([128, 512], F32, tag="av", name="avB")

                    qk_next = qk_mm(c6, 0, q0, QB)
                    for ki in range(KT):
                        kwid = kw_of(ki)
                        qk = qk_next
                        if V_SERIAL:
                            pass
                        elif qbi == 0:
                            if c6 == 0:
                                if ki + 2 < KT:
                                    emit_vproj(ki + 2)
                                if ki in (1, 4, 7):
                                    a = (ki + 2) // 3 * QB
                                    emit_proj(kw, tT, kT, kb_t, 0, a,
                                              QB if a < NQB * QB else QR)
                                if pend and ki in (3, 5, 9, 11, 12):
                                    pend.pop(0)()
                            elif pend and ki % 2 == 1:
                                pend.pop(0)()
                        elif filler and (ki % 4 == 3):
                            filler_emit(1)
                        if ki + 1 < KT:
                            qk_next = qk_mm(c6, ki + 1, q0, QB)
                        ex = P_pool.tile([128, 2, 512], BF16, tag="P")
                        nc.scalar.activation(
                            ex[:kwid, :, :],
                            qk[:kwid, :].rearrange("p (h q) -> p h q", h=2),
                            mybir.ActivationFunctionType.Exp,
                        )
                        if DBG and qbi == 0 and c6 == 0 and ki == 0:
                            nc.sync.dma_start(dbg_P[:], ex[:, :, :])
                        for hh in range(2):
                            nc.tensor.matmul(
                                (avA if hh == 0 else avB)[:, 0:QB],
                                vP[ki][:kwid, hh, c6, :],
                                ex[:kwid, hh, :],
                                start=(ki == 0), stop=(ki == KT - 1),
                            )
                    av_evac(c6, avA, avB, q0, QB)

            # ---- rump q block (32 cols): batch QK psum across all ki
            q0 = NQB * QB
            with nc.named_scope("rump"):
                for c6 in (range(NCH) if V_RUMP else []):
                    if not V_QALL:
                        emit_proj(qw, sT, qT, qb_t, c6, q0, QR)
                    avA = av_psum.tile([128, 512], F32, tag="av", name="avA")
                    avB = av_psum.tile([128, 512], F32, tag="av", name="avB")
                    qk = qk_psum.tile([128, 1024], F32, tag="qk", name="qkr")
                    # head-major, ki padded to 16: head A fills psum bank 0,
                    # head B bank 1 (concurrent pair must hit distinct banks)
                    qkv = qk[:, :].rearrange("p (h k q) -> p h k q", h=2, k=16)
                    for ki in range(KT):
                        kwid = kw_of(ki)
                        ksl = slice(ki * 128, ki * 128 + kwid)
                        nc.tensor.matmul(
                            qkv[:kwid, 0, ki, :], kT[c6][0:64, ksl],
                            qT[c6][0:64, q0:q0 + QR],
                            start=True, stop=True, tile_position=(0, 0),
                        )
                        nc.tensor.matmul(
                            qkv[:kwid, 1, ki, :], kT[c6][64:128, ksl],
                            qT[c6][64:128, q0:q0 + QR],
                            start=True, stop=True, tile_position=(64, 0),
                        )
                    ex = P_pool.tile([128, 2, KT, QR], BF16, tag="Pr")
                    for hh in range(2):
                        nc.scalar.activation(
                            ex[:, hh, 0:12, :], qkv[:, hh, 0:12, :],
                            mybir.ActivationFunctionType.Exp,
                        )
                        nc.scalar.activation(
                            ex[0:KR, hh, 12, :], qkv[0:KR, hh, 12, :],
                            mybir.ActivationFunctionType.Exp,
                        )
                    filler_emit(2)
                    for ki in range(KT):
                        kwid = kw_of(ki)
                        for hh in range(2):
                            nc.tensor.matmul(
                                (avA if hh == 0 else avB)[:, 0:QR],
                                vP[ki][:kwid, hh, c6, :],
                                ex[:kwid, hh, ki, :],
                                start=(ki == 0), stop=(ki == KT - 1),
                            )
                    av_evac(c6, avA, avB, q0, QR)
                    filler.append(
                        (lambda c=c6, a=(NQB - 1) * QB:
                         emit_outproj(c, a, QB)))

            if DBG:
                nc.sync.dma_start(dbg_sT[:], sT[0][:])
                for c in range(NCH):
                    nc.sync.dma_start(dbg_qT[c], qT[c][:])
                nc.sync.dma_start(dbg_kT[:], kT[0][:])
                nc.sync.dma_start(dbg_vP[:], vP[0][:])
                for c in range(NCH):
                    nc.sync.dma_start(dbg_OT[c], OT[c][:])
            # ---- drain remaining filler + final out-proj slices
            with nc.named_scope("tail"):
                filler_emit(len(filler))
                if V_RUMP:
                    for c6 in range(NCH):
                        emit_outproj(c6, NQB * QB, QR)

    nc.finalize()
    return nc


def _install_axon_ntff_shim():
    if "antenv.axon_hooks" in sys.modules:
        return
    mod = types.ModuleType("antenv.axon_hooks")
    mod._hook = None
    mod.set_axon_ntff_profile_hook = lambda h: setattr(mod, "_hook", h)
    mod.get_axon_ntff_profile_hook = lambda: mod._hook
    sys.modules["antenv.axon_hooks"] = mod
    try:
        import antenv

        antenv.axon_hooks = mod
        from trn_agent_boot.trn_boot import _ntff_profile_via_ctypes

        hook = _ntff_profile_via_ctypes("/opt/axon/libaxon_pjrt.so")
        if hook is not None:
            mod.set_axon_ntff_profile_hook(hook)
    except Exception:
        pass


def prep_inputs(s_x, t_x, clip_space_pos, vmae_space_pos, clip_temporal_pos,
                vmae_temporal_pos, q_w, q_b, kv_w, kv_b, proj_w, proj_b):
    """Host-side sharding/layout prep. Returns list of 8 per-core input maps."""
    f = np.float32
    bf = ml_dtypes.bfloat16
    sp_s = np.ascontiguousarray(np.asarray(clip_space_pos).T).astype(bf)
    tp_s = np.ascontiguousarray(np.asarray(clip_temporal_pos).T).astype(bf)
    sp_t = np.ascontiguousarray(np.asarray(vmae_space_pos).T).astype(bf)
    tp_t = np.ascontiguousarray(np.asarray(vmae_temporal_pos).T).astype(bf)
    q_wT = np.ascontiguousarray(np.asarray(q_w).T * SCALE).astype(bf)
    k_wT = np.ascontiguousarray(np.asarray(kv_w)[:DIM].T).astype(bf)
    # v weight rows reordered: [even heads' v dims | odd heads' v dims]
    v_w = np.asarray(kv_w)[DIM:]         # [768 out, 768 in]
    v_b = np.asarray(kv_b)[DIM:]
    order = np.concatenate([
        np.arange(DIM).reshape(H, DH)[0::2].reshape(-1),
        np.arange(DIM).reshape(H, DH)[1::2].reshape(-1),
    ])
    v_wT = np.ascontiguousarray(v_w[order].T).astype(bf)
    v_br = np.ascontiguousarray(
        np.broadcast_to(v_b[order].reshape(1, DIM), (128, DIM)), dtype=f)
    proj_wT = np.ascontiguousarray(np.asarray(proj_w).T).astype(bf)
    q_b2 = np.ascontiguousarray(
        (np.asarray(q_b) * SCALE).reshape(NCH, 128).T, dtype=f)
    k_b2 = np.ascontiguousarray(
        np.asarray(kv_b)[:DIM].reshape(NCH, 128).T, dtype=f)
    p_b2 = np.ascontiguousarray(np.asarray(proj_b).reshape(NCH, 128).T, dtype=f)

    in_maps = []
    for b in range(B):
        s_slice = np.asarray(s_x)[:, b * TS:(b + 1) * TS, :]  # (196, 8, 768)
        t_slice = np.asarray(t_x)[1:, b * T:(b + 1) * T, :]   # (196, 8, 768)
        s_xT = np.ascontiguousarray(
            s_slice.transpose(2, 0, 1).reshape(DIM, NT)).astype(bf)
        t_xT = np.ascontiguousarray(
            t_slice.transpose(2, 0, 1).reshape(DIM, NT)).astype(bf)
        in_maps.append({
            "s_xT": s_xT, "t_xT": t_xT,
            "sp_s": sp_s, "tp_s": tp_s, "sp_t": sp_t, "tp_t": tp_t,
            "q_wT": q_wT, "k_wT": k_wT, "v_wT": v_wT, "proj_wT": proj_wT,
            "q_b2": q_b2, "k_b2": k_b2, "p_b2": p_b2, "v_br": v_br,
        })
    return in_maps


def unshard_output(results):
    """results: list of 8 dicts with 'outT' [768, 1568] -> (196, 64, 768)."""
    out = np.empty((APATCH, B * TS, DIM), dtype=np.float32)
    for b in range(B):
        o = results[b]["outT"].reshape(DIM, APATCH, TS)
        out[:, b * TS:(b + 1) * TS, :] = o.transpose(1, 2, 0)
    return out


def kernel(**inputs):
    _install_axon_ntff_shim()
    in_maps = prep_inputs(**inputs)
    if "nc" not in _NC_CACHE:
        _NC_CACHE["nc"] = build_nc()
    nc = _NC_CACHE["nc"]
    res = run_bass_kernel_spmd(nc, in_maps, core_ids=list(range(B)))
    return unshard_output(res.results)


if __name__ == "__main__":
    rng = np.random.default_rng(0)
    fake = {
        "s_x": rng.standard_normal((APATCH, B * TS, DIM), dtype=np.float32),
        "t_x": rng.standard_normal((VP + 1, B * T, DIM), dtype=np.float32),
        "clip_space_pos": SCALE * rng.standard_normal((APATCH, DIM), dtype=np.float32),
        "vmae_space_pos": SCALE * rng.standard_normal((VP, DIM), dtype=np.float32),
        "clip_temporal_pos": SCALE * rng.standard_normal((TS, DIM), dtype=np.float32),
        "vmae_temporal_pos": SCALE * rng.standard_normal((T, DIM), dtype=np.float32),
        "q_w": (0.02 * rng.standard_normal((DIM, DIM))).astype(np.float32),
        "q_b": np.zeros(DIM, np.float32),
        "kv_w": (0.02 * rng.standard_normal((2 * DIM, DIM))).astype(np.float32),
        "kv_b": np.zeros(2 * DIM, np.float32),
        "proj_w": (0.02 * rng.standard_normal((DIM, DIM))).astype(np.float32),
        "proj_b": np.zeros(DIM, np.float32),
    }
    out = kernel(**fake)
    print("out", out.shape, out.dtype)
